# revision 1
# baseline (speedup 1.0000x reference)
"""GAT forward kernel for Trainium2 (8 NeuronCores, Bass/Tile).

Reference computation (dense form):
    adj = densify(A); Wh = X @ Ws; e = leaky_relu(Wh@a1 + (Wh@a2).T, 0.15)
    att = softmax(where(adj>0, e, -9e15), axis=1); out = elu(att @ Wh)

This kernel exploits sparsity: only ~524K edges out of 16384^2 matter.
Because |e| <= ~16 (bounded inputs), softmax needs no max-subtraction:
    w_e = exp(leaky(s_src + t_dst));  out_i = elu(sum_e w_e Wh_dst / sum_e w_e)
with exp(-9e15) == 0 handled by simply not summing non-edges, and duplicate
edges deduplicated on the host (reference only uses adj > 0).

Sharding: rows (softmax queries) split 2048/core across 8 cores. Each core:
  P1: computes Wh = X @ Ws (PE transpose + matmul, bf16), s = Wh@a1,
      t = Wh@a2 (f32) for ALL nodes, writes a DRAM table row
      j = [t_j f32, s_j f32, Wh_j bf16] (256B).
  P3: dma_gathers table rows by edge dst (256B each, one multi-packet call
      per 128-row block) and per 8-edge cell rows by edge src; computes
      w on DVE/ACT; segment-aggregates per 128-row block via one-hot PE
      matmuls in bf16:
          acc[128,65] += onehot(srcrel)[128e,128r].T @ (w * [Wh_dst, 1])
      then out = elu(U / Z) and writes its 2048 output rows.

Host prep packs edges into per-(core,block) buckets with 8-slot cells
padded to a cross-core uniform chunk count so all 8 cores run the same
program (SPMD).
"""
import os
import sys

if "/opt/trn_rl_repo" not in sys.path:
    sys.path.insert(0, "/opt/trn_rl_repo")

_ABL = set(os.environ.get("GAT_ABLATE", "").split(","))

from contextlib import ExitStack

import numpy as np

import concourse.bass as bass
import concourse.tile as tile
from concourse import bacc, mybir
from concourse.bass_utils import run_bass_kernel_spmd
from concourse.masks import make_identity

N = 16384          # nodes
F = 128            # input features
D = 64             # embedding dim
NCORES = 8
R = N // NCORES    # rows per core (2048)
NB = R // 128      # row blocks per core (16)
NBG = N // 128     # global node blocks (128)
TW = 64            # table row width in f32 slots (256 bytes)
CELL = 8           # slots per cell (one src row per cell)
dt = mybir.dt


# ---------------------------------------------------------------- host prep
def _prep_edges(A):
    """Dedup edges, bucket by (core, block) with each row's edges padded to a
    multiple of CELL (so every CELL-slot "cell" belongs to one src row), then
    pad blocks to cross-core uniform chunk counts Kb. Returns per-core index /
    srcrel / cell arrays and the shared Kb."""
    src_all = np.asarray(A[0], dtype=np.int64)
    dst_all = np.asarray(A[1], dtype=np.int64)
    keys = np.unique(src_all * N + dst_all)     # dedup + sort by (src, dst)
    src = (keys // N).astype(np.int32)
    dst = (keys % N).astype(np.int32)

    deg = np.bincount(src, minlength=N)
    assert deg.min() > 0, (
        "empty rows present; this kernel assumes every row has >=1 edge"
    )
    degc = ((deg + CELL - 1) // CELL) * CELL     # CELL-aligned row sizes
    gb = np.arange(N) >> 7
    cntc = np.bincount(gb, weights=degc, minlength=NBG).astype(np.int64)
    cntc = cntc.reshape(NCORES, NB)
    Kb = np.maximum((cntc.max(axis=0) + 127) // 128, 1)          # [NB]
    S = int(Kb.sum()) * 128                      # slots per core
    offs = np.concatenate([[0], np.cumsum(Kb)]) * 128  # slot offset per block
    # cells per block, padded to 128-cell granularity for the gather
    ncell = [int(k) * (128 // CELL) for k in Kb]
    ncellp = [((n + 127) // 128) * 128 for n in ncell]
    cell_offs = np.concatenate([[0], np.cumsum(ncellp)])
    SC = int(cell_offs[-1])                      # padded cells per core

    row_start = np.concatenate([[0], np.cumsum(deg)])

    dsti = np.zeros((NCORES, S), np.int16)       # table idx for dst gather
    srcrel = np.full((NCORES, S), -1.0, np.float32)  # row-in-block, -1 = pad
    cellsrc = np.zeros((NCORES, SC), np.int16)   # global src row per cell
    for c in range(NCORES):
        for b in range(NB):
            rows = np.arange((c * NB + b) * 128, (c * NB + b) * 128 + 128)
            pos = offs[b]
            for r in rows:
                d = int(deg[r])
                lo = row_start[r]
                dsti[c, pos:pos + d] = dst[lo:lo + d]
                srcrel[c, pos:pos + d] = float(r & 127)
                ncw = int(degc[r])
                cbase = cell_offs[b] + (pos - offs[b]) // CELL
                cellsrc[c, cbase:cbase + ncw // CELL] = r
                pos += ncw
            assert pos <= offs[b + 1]

    # Per-cell s comes from on-chip PE selection (no DMA gather): for each
    # 128-cell span g of block b, PM[:, g, :] is a one-hot [row-in-block i,
    # cell-slot p] matrix with PM[i, p] = 1 iff cell (b, g*128+p) belongs to
    # row base_b + i. colsel picks this core's 16 columns out of the s_all
    # staging (one-hot over global node blocks).
    import ml_dtypes
    NMTOT = sum(n // 128 for n in ncellp)
    cores = []

    def wrap(x):
        n = x.shape[0]
        w = x.reshape(n // 16, 16).T             # [16, n/16]
        return np.tile(w, (8, 1)).copy()         # [128, n/16]

    for c in range(NCORES):
        PMs = np.zeros((128, NMTOT, 128), ml_dtypes.bfloat16)
        g = 0
        for b in range(NB):
            base = (c * NB + b) * 128
            for m in range(ncellp[b] // 128):
                cs = cellsrc[c, cell_offs[b] + m * 128: cell_offs[b] + (m + 1) * 128]
                rel = cs.astype(np.int64) - base
                valid = (rel >= 0) & (rel < 128)
                PMs[rel[valid], g, np.arange(128)[valid]] = 1.0
                g += 1
        assert g == NMTOT
        colsel = np.zeros((128, NB), np.float32)
        colsel[np.arange(c * NB, (c + 1) * NB), np.arange(NB)] = 1.0
        cores.append({
            "comb": wrap(dsti[c]),
            "srcrel": srcrel[c].reshape(S // 128, 128).T.copy(),  # [128, S/128]
            "PMs": PMs,
            "colsel": colsel,
        })
    return cores, [int(k) for k in Kb], S, [int(x) for x in ncellp]


_qctr = [0]


def _q():
    # Tile assigns DMA-SW sem lanes round-robin in its own issue order; a
    # queue rotation here desyncs from that lane->queue pinning (sim flags
    # cross-queue sem updates), so pin everything to queue 0.
    return 0


# ---------------------------------------------------------------- device IR
def _build(Kb, S, ncellp):
    _qctr[0] = 0
    SC = sum(ncellp)
    NMTOT = SC // 128
    nc = bacc.Bacc("TRN2", target_bir_lowering=False, debug=False,
                   enable_asserts=False, num_devices=NCORES,
                   num_swdge_queues=4)
    XT_d = nc.dram_tensor("XT", [F, N], dt.bfloat16, kind="ExternalInput").ap()
    Ws_d = nc.dram_tensor("Ws", [F, D], dt.float32, kind="ExternalInput").ap()
    WsT_d = nc.dram_tensor("WsT", [D, F], dt.float32, kind="ExternalInput").ap()
    apair_d = nc.dram_tensor("apair", [D, 2], dt.float32, kind="ExternalInput").ap()
    comb_d = nc.dram_tensor("comb", [128, S // 16], dt.int16, kind="ExternalInput").ap()
    PMs_d = nc.dram_tensor("PMs", [128, NMTOT, 128], dt.bfloat16, kind="ExternalInput").ap()
    colsel_d = nc.dram_tensor("colsel", [128, NB], dt.float32, kind="ExternalInput").ap()
    ident_d = nc.dram_tensor("ident", [128, 128], dt.float32, kind="ExternalInput").ap()
    srel_d = nc.dram_tensor("srcrel", [128, S // 128], dt.float32, kind="ExternalInput").ap()
    sel16_d = nc.dram_tensor("sel16", [128, CELL], dt.float32, kind="ExternalInput").ap()
    E16_d = nc.dram_tensor("E16", [128, 128], dt.float32, kind="ExternalInput").ap()
    iotaf_d = nc.dram_tensor("iotaf", [128, 128], dt.float32, kind="ExternalInput").ap()
    out_d = nc.dram_tensor("out", [R, D], dt.float32, kind="ExternalOutput").ap()

    with tile.TileContext(nc) as tc, ExitStack() as ctx:
        cpool = ctx.enter_context(tc.tile_pool(name="const", bufs=1))
        dram = ctx.enter_context(tc.tile_pool(name="dram", bufs=1, space="DRAM"))
        xpool = ctx.enter_context(tc.tile_pool(name="x", bufs=2))
        xbfpool = ctx.enter_context(tc.tile_pool(name="xbf", bufs=2))
        xtpool = ctx.enter_context(tc.tile_pool(name="xt", bufs=3))
        twpool = ctx.enter_context(tc.tile_pool(name="tw", bufs=3))
        tmppool = ctx.enter_context(tc.tile_pool(name="tmp", bufs=2))
        # PSUM budget (8 banks): P1 Wh pool 4x1 banks; P3 acc 2x1;
        # s-expand 2x1.
        ps_p1 = ctx.enter_context(tc.tile_pool(name="ps_p1", bufs=2, space="PSUM"))
        ps_sm = ctx.enter_context(tc.tile_pool(name="ps_sm", bufs=2, space="PSUM"))
        ps_se = ctx.enter_context(tc.tile_pool(name="ps_se", bufs=1, space="PSUM"))
        gpool = ctx.enter_context(tc.tile_pool(name="gat", bufs=4))
        spool = ctx.enter_context(tc.tile_pool(name="sg", bufs=4))
        wpool = ctx.enter_context(tc.tile_pool(name="w", bufs=2))
        Gpool = ctx.enter_context(tc.tile_pool(name="G", bufs=3))
        ohpool = ctx.enter_context(tc.tile_pool(name="oh", bufs=2))
        epool = ctx.enter_context(tc.tile_pool(name="ep", bufs=2))

        tabTW = dram.tile([N, TW], dt.float32)   # [t, s, Wh(64), garbage pad]

        # ---- constants
        iota_f = cpool.tile([128, 128], dt.float32)
        nc.sync.dma_start(iota_f[:], iotaf_d)
        ws_t = cpool.tile([F, D], dt.float32)
        nc.sync.dma_start(ws_t[:], Ws_d)
        wsT_t = cpool.tile([D, F], dt.float32)
        nc.sync.dma_start(wsT_t[:], WsT_d)
        apair_t = cpool.tile([D, 2], dt.float32)
        nc.sync.dma_start(apair_t[:], apair_d)
        # Wse = [Ws@a2 | Ws@a1 | Ws] bf16: one 66-col rhs so each P1 matmul
        # yields [t, s, Wh] directly in table order.
        wsa_ps = ps_se.tile([128, 2], dt.float32, space="PSUM", tag="se")
        nc.tensor.matmul(wsa_ps[:], lhsT=wsT_t[:], rhs=apair_t[:],
                         start=True, stop=True)
        Wse = cpool.tile([F, 2 + D], dt.bfloat16)
        nc.vector.tensor_copy(Wse[:, 0:2], wsa_ps[:])
        nc.vector.tensor_copy(Wse[:, 2:2 + D], ws_t[:])
        comb_t = cpool.tile([128, S // 16], dt.int16)
        nc.sync.dma_start(comb_t[:], comb_d)
        PMs_t = cpool.tile([128, NMTOT, 128], dt.bfloat16)
        nc.sync.dma_start(PMs_t[:], PMs_d)
        colsel_t = cpool.tile([128, NB], dt.float32)
        nc.sync.dma_start(colsel_t[:], colsel_d)
        ident_t = cpool.tile([128, 128], dt.float32)
        nc.sync.dma_start(ident_t[:], ident_d)
        s_all = cpool.tile([128, 128], dt.float32)
        srel_t = cpool.tile([128, S // 128], dt.float32)
        nc.sync.dma_start(srel_t[:], srel_d)
        sel16_t = cpool.tile([128, CELL], dt.float32)
        nc.sync.dma_start(sel16_t[:], sel16_d)
        E16_t = cpool.tile([128, 128], dt.float32)
        nc.sync.dma_start(E16_t[:], E16_d)

        # ---- P1: build table row j = [t_j, s_j, Wh_j(64), pad] for all nodes
        # XT is the host-transposed bf16 X: one matmul per 128-node chunk
        # yields [t, s, Wh] columns at once (rhs = Wse).
        XT_v = XT_d.rearrange("f (q k n) -> q f k n", k=8, n=128)  # [16, 128, 8, 128]
        tab_v = tabTW[:].rearrange("(q k p) w -> p q k w", p=128, k=8)
        for q in range(16):                       # groups of 8 node blocks
            xtq = xtpool.tile([128, 8, 128], dt.bfloat16)
            nc.scalar.dma_start(xtq[:], XT_v[q])
            tw = twpool.tile([128, 8, 34], dt.float32)
            if "nop1" not in _ABL:
                wh_psA = ps_p1.tile([128, 4, 2 + D], dt.float32, space="PSUM", tag="p1a")
                wh_psB = ps_p1.tile([128, 4, 2 + D], dt.float32, space="PSUM", tag="p1b")
                for k in range(8):
                    ps = wh_psA if k < 4 else wh_psB
                    nc.tensor.matmul(ps[:, k % 4, :], lhsT=xtq[:, k, :],
                                     rhs=Wse[:], start=True, stop=True)
                nc.vector.tensor_copy(tw[:, 0:4, 0:2], wh_psA[:, :, 0:2])
                nc.vector.tensor_copy(tw[:, 4:8, 0:2], wh_psB[:, :, 0:2])
                nc.vector.tensor_copy(s_all[:, q * 8:q * 8 + 4], wh_psA[:, :, 1])
                nc.vector.tensor_copy(s_all[:, q * 8 + 4:q * 8 + 8], wh_psB[:, :, 1])
                nc.vector.tensor_copy(tw[:, 0:4, 2:34].bitcast(dt.bfloat16),
                                      wh_psA[:, :, 2:2 + D])
                nc.vector.tensor_copy(tw[:, 4:8, 2:34].bitcast(dt.bfloat16),
                                      wh_psB[:, :, 2:2 + D])
            # write rows (q*8+k)*128+p, cols 0:34 (pad cols stay garbage --
            # they are gathered but never read by any compute)
            nc.scalar.dma_start(tab_v[:, q, :, 0:34], tw[:])

        # ---- s_local: this core's per-block s vectors, from s_all via one
        # transpose + colsel matmul; split hi/lo bf16 so the PM matmuls stay
        # near-f32 exact.
        sT_ps = ps_p1.tile([128, 128], dt.float32, space="PSUM", tag="p1a")
        nc.tensor.transpose(sT_ps[:], s_all[:], ident_t[:])
        sT = cpool.tile([128, 128], dt.float32)
        nc.vector.tensor_copy(sT[:], sT_ps[:])
        sloc_ps = ps_se.tile([128, NB], dt.float32, space="PSUM", tag="se")
        nc.tensor.matmul(sloc_ps[:], lhsT=sT[:], rhs=colsel_t[:],
                         start=True, stop=True)
        sloc_hl = cpool.tile([128, NB, 2], dt.bfloat16)
        nc.vector.tensor_copy(sloc_hl[:, :, 0], sloc_ps[:])
        nc.vector.tensor_sub(sloc_hl[:, :, 1], sloc_ps[:], sloc_hl[:, :, 0])

        # ---- pre-build one-hots for the last blocks so the tail isn't
        # DVE-serialized behind earlier blocks' gather-gated work
        PRE = 6
        Kmax = max(Kb)
        sl_offs = [0]
        for b in range(NB):
            sl_offs.append(sl_offs[-1] + Kb[b] * 128)
        ohpre = cpool.tile([128, PRE, Kmax, 128], dt.bfloat16)
        for i, b in enumerate(range(NB - PRE, NB)):
            Kpre = Kb[b]
            nc.vector.tensor_tensor(
                out=ohpre[:, i, 0:Kpre, :],
                in0=iota_f[:, None, :].to_broadcast([128, Kpre, 128]),
                in1=srel_t[:, sl_offs[b] // 128:sl_offs[b] // 128 + Kpre, None]
                    .to_broadcast([128, Kpre, 128]),
                op=mybir.AluOpType.is_equal)

        # ---- P3: per-block gather + weight + one-hot aggregate + epilogue
        tab_ap = tabTW[:]                                    # [N, 128] rows
        outstage = cpool.tile([128, NB, D], dt.float32)
        off = 0          # slot offset
        sl_off = 0       # same (kept for clarity below)
        gsp = 0          # global PM span index
        for b in range(NB):
            K = Kb[b]
            n_idx = K * 128
            ncp = ncellp[b]                      # padded cell count (x128)
            nm = ncp // 128                      # 128-cell spans
            gat = gpool.tile([128, K, TW], dt.float32)
            if "init" in _ABL:
                nc.vector.memzero(gat[:])
            if "nogat" not in _ABL:
                nc.gpsimd.dma_gather(
                    out_ap=gat[:], in_ap=tab_ap,
                    idxs_ap=comb_t[:, off // 16:(off + n_idx) // 16],
                    num_idxs=n_idx, num_idxs_reg=n_idx,
                    elem_size=TW, queue_num=_q(), single_packet=False,
                )
            # one-hot of srcrel vs row-in-block (bf16); last PRE blocks use
            # the pre-built tiles
            if b >= NB - PRE:
                oh = ohpre[:, b - (NB - PRE), 0:K, :]
            else:
                oht = ohpool.tile([128, K, 128], dt.bfloat16)
                nc.vector.tensor_tensor(
                    out=oht[:],
                    in0=iota_f[:, None, :].to_broadcast([128, K, 128]),
                    in1=srel_t[:, sl_off // 128:sl_off // 128 + K, None]
                        .to_broadcast([128, K, 128]),
                    op=mybir.AluOpType.is_equal)
                oh = oht[:]
            # s per cell via PM one-hot matmuls (no DMA), then expand to the
            # edge layout via constant matmuls (CPC = 128//CELL cells/chunk):
            #   s_edge[p, CELL*m+cl] = cellval[CPC*cl + p//CELL, m]
            #     = sum_q E16[q, p] * (cellval[q, m] * sel16[q, cl])
            sc_ps = ps_se.tile([128, nm], dt.float32, space="PSUM", tag="sc")
            for m in range(nm):
                nc.tensor.matmul(sc_ps[:, m:m + 1], lhsT=PMs_t[:, gsp + m, :],
                                 rhs=sloc_hl[:, b, 0:1], start=True, stop=False)
                nc.tensor.matmul(sc_ps[:, m:m + 1], lhsT=PMs_t[:, gsp + m, :],
                                 rhs=sloc_hl[:, b, 1:2], start=False, stop=True)
            s_ps = ps_se.tile([128, nm * CELL], dt.float32, space="PSUM", tag="se")
            for m in range(nm):
                rhsm = wpool.tile([128, CELL], dt.float32, tag="rhsm")
                nc.vector.tensor_mul(
                    rhsm[:], sel16_t[:],
                    sc_ps[:, m:m + 1].to_broadcast([128, CELL]))
                nc.tensor.matmul(s_ps[:, m * CELL:(m + 1) * CELL], lhsT=E16_t[:],
                                 rhs=rhsm[:], start=True, stop=True)
            # w = exp(leaky(s + t))
            e_t = wpool.tile([128, K], dt.float32, tag="e")
            nc.vector.tensor_add(e_t[:], s_ps[:, 0:K], gat[:, 0:K, 0])
            lk = wpool.tile([128, K], dt.float32, tag="lk")
            nc.vector.scalar_tensor_tensor(
                out=lk[:], in0=e_t[:], scalar=0.15, op0=mybir.AluOpType.mult,
                in1=e_t[:], op1=mybir.AluOpType.max)
            w_t = wpool.tile([128, K], dt.float32, tag="wt")
            nc.scalar.activation(w_t[:], lk[:], mybir.ActivationFunctionType.Exp)
            # G = [w * Wh_dst, w]  (bf16 for full-rate PE)
            G = Gpool.tile([128, K, D + 1], dt.bfloat16)
            nc.vector.tensor_mul(G[:, :, 0:D], gat[:, 0:K, 2:34].bitcast(dt.bfloat16),
                                 w_t[:, :, None].to_broadcast([128, K, D]))
            nc.vector.tensor_copy(G[:, :, D], w_t[:])
            # aggregate
            acc = ps_sm.tile([128, D + 1], dt.float32, space="PSUM", tag="sm")
            nmm = 1 if "nomm" in _ABL else K
            for c in range(nmm):
                nc.tensor.matmul(acc[:], lhsT=oh[:, c, :], rhs=G[:, c, :],
                                 start=(c == 0), stop=(c == nmm - 1))
            # epilogue: out = elu(U / Z)
            zg = epool.tile([128, 1], dt.float32, tag="zg")
            nc.vector.tensor_scalar_max(zg[:], acc[:, D:D + 1], 1e-30)
            zr = epool.tile([128, 1], dt.float32, tag="zr")
            nc.vector.reciprocal(zr[:], zg[:])
            x = epool.tile([128, D], dt.float32, tag="x")
            nc.vector.tensor_scalar_mul(x[:], acc[:, 0:D], zr[:])
            mn = epool.tile([128, D], dt.float32, tag="mn")
            nc.vector.tensor_scalar_min(mn[:], x[:], 0.0)
            em = epool.tile([128, D], dt.float32, tag="em")
            nc.scalar.activation(em[:], mn[:], mybir.ActivationFunctionType.Exp)
            rl = epool.tile([128, D], dt.float32, tag="rl")
            nc.vector.tensor_scalar_max(rl[:], x[:], 0.0)
            nc.vector.scalar_tensor_tensor(
                out=outstage[:, b, :], in0=em[:], scalar=-1.0,
                op0=mybir.AluOpType.add, in1=rl[:], op1=mybir.AluOpType.add)
            off += n_idx
            sl_off += n_idx
            gsp += nm

        out_v = out_d.rearrange("(b p) d -> p b d", p=128)   # [128, NB, D]
        nc.sync.dma_start(out_v, outstage[:])
    nc.compile()
    return nc


_cache = {}


def _get_program(Kb, S, ncellp):
    key = (tuple(Kb), S, tuple(ncellp), tuple(sorted(_ABL)))
    if key not in _cache:
        _cache[key] = _build(Kb, S, ncellp)
    return _cache[key]


def make_in_maps(A, X, Ws, a):
    """Host-side sharding: returns (nc, in_maps)."""
    import ml_dtypes
    X = np.asarray(X, dtype=np.float32)
    Ws = np.ascontiguousarray(np.asarray(Ws, dtype=np.float32))
    a = np.asarray(a, dtype=np.float32).reshape(2 * D)
    XT = np.ascontiguousarray(X.T).astype(ml_dtypes.bfloat16)
    WsT = np.ascontiguousarray(Ws.T)
    apair = np.stack([a[D:], a[:D]], axis=1).astype(np.float32)  # [D, 2] = [a2|a1]
    q = np.arange(128)
    CPC = 128 // CELL
    sel16 = (q[:, None] // CPC == np.arange(CELL)[None, :]).astype(np.float32)
    E16 = (q[:, None] % CPC == q[None, :] // CELL).astype(np.float32)
    iotaf = np.tile(np.arange(128, dtype=np.float32)[None, :], (128, 1))
    cores, Kb, S, ncellp = _prep_edges(A)
    nc = _get_program(Kb, S, ncellp)
    in_maps = [
        {"XT": XT, "Ws": Ws, "WsT": WsT, "apair": apair, "sel16": sel16,
         "E16": E16, "iotaf": iotaf,
         "comb": c["comb"], "srcrel": c["srcrel"], "PMs": c["PMs"],
         "colsel": c["colsel"], "ident": np.eye(128, dtype=np.float32)}
        for c in cores
    ]
    return nc, in_maps


def kernel(A, X, Ws, a):
    nc, in_maps = make_in_maps(A, X, Ws, a)
    res = run_bass_kernel_spmd(nc, in_maps, core_ids=list(range(NCORES)),
                               trace=False)
    return np.concatenate([r["out"] for r in res.results], axis=0)



# revision 2
# speedup vs baseline: 2.7097x; 2.7097x over previous
"""GAT forward kernel for Trainium2 (8 NeuronCores, Bass/Tile).

Reference computation (dense form):
    adj = densify(A); Wh = X @ Ws; e = leaky_relu(Wh@a1 + (Wh@a2).T, 0.15)
    att = softmax(where(adj>0, e, -9e15), axis=1); out = elu(att @ Wh)

Sparse form (only ~524K of 16384^2 entries matter; |e| <= ~16 so softmax
needs no max-subtraction):
    w_e = exp(leaky(s_src + t_dst));  out_i = elu(sum_e w_e Wh_dst / sum_e w_e)

Sharding: rows (softmax queries) split 2048/core across 8 cores.

Per-edge data movement strategy: the host pre-gathers X[dst_e] for every
edge slot into a contiguous, feature-major bf16 tensor XgT [128F, S].
On device, one matmul per 128-slot chunk (lhsT = XgT chunk, rhs =
[Ws@a2 | Ws]) produces [t_dst, Wh_dst] per slot directly in slot-partition
layout -- no DMA gather, no GPSIMD descriptor generation, no DRAM table.

Per-slot s_src comes from the cell trick: edges padded per-row to CELL=8
slots; per 128-cell span a host-built one-hot PM picks s values by PE
matmul, then constant matmuls expand cells to slots.

Aggregation per block b (128 rows):
    acc[128,65] += onehot(srcrel)[128slot,128row].T @ (w * [Wh_dst, 1])
then out = elu(U / Z).

Host prep packs edges into per-(core,block) buckets with 8-slot cells
padded to a cross-core uniform chunk count so all 8 cores run the same
program (SPMD).
"""
import os
import sys

if "/opt/trn_rl_repo" not in sys.path:
    sys.path.insert(0, "/opt/trn_rl_repo")

_ABL = set(os.environ.get("GAT_ABLATE", "").split(","))

from contextlib import ExitStack

import numpy as np

import concourse.bass as bass
import concourse.tile as tile
from concourse import bacc, mybir
from concourse.bass_utils import run_bass_kernel_spmd

N = 16384          # nodes
F = 128            # input features
D = 64             # embedding dim
NCORES = 8
R = N // NCORES    # rows per core (2048)
NB = R // 128      # row blocks per core (16)
NBG = N // 128     # global node blocks (128)
CELL = 8           # slots per cell (one src row per cell)
SUB = 7            # A1 chunks per PSUM sub-batch (7*65*4B < 2KB bank)
dt = mybir.dt


# ---------------------------------------------------------------- host prep
def _prep_edges(A):
    """Dedup edges, bucket by (core, block) with each row's edges padded to a
    multiple of CELL (so every CELL-slot "cell" belongs to one src row), then
    pad blocks to cross-core uniform chunk counts Kb. Returns per-core index /
    srcrel / cell arrays and the shared Kb."""
    src_all = np.asarray(A[0], dtype=np.int64)
    dst_all = np.asarray(A[1], dtype=np.int64)
    keys = np.unique(src_all * N + dst_all)     # dedup + sort by (src, dst)
    src = (keys // N).astype(np.int32)
    dst = (keys % N).astype(np.int32)

    deg = np.bincount(src, minlength=N)
    assert deg.min() > 0, (
        "empty rows present; this kernel assumes every row has >=1 edge"
    )
    degc = ((deg + CELL - 1) // CELL) * CELL     # CELL-aligned row sizes
    gb = np.arange(N) >> 7
    cntc = np.bincount(gb, weights=degc, minlength=NBG).astype(np.int64)
    cntc = cntc.reshape(NCORES, NB)
    Kb = np.maximum((cntc.max(axis=0) + 127) // 128, 1)          # [NB]
    S = int(Kb.sum()) * 128                      # slots per core
    offs = np.concatenate([[0], np.cumsum(Kb)]) * 128  # slot offset per block
    # cells per block, padded to 128-cell granularity for the PM matmuls
    ncell = [int(k) * (128 // CELL) for k in Kb]
    ncellp = [((n + 127) // 128) * 128 for n in ncell]
    cell_offs = np.concatenate([[0], np.cumsum(ncellp)])
    SC = int(cell_offs[-1])                      # padded cells per core

    row_start = np.concatenate([[0], np.cumsum(deg)])

    dsti = np.zeros((NCORES, S), np.int32)       # global dst node per slot
    srcrel = np.full((NCORES, S), -1.0, np.float32)  # row-in-block, -1 = pad
    cellsrc = np.zeros((NCORES, SC), np.int16)   # global src row per cell
    for c in range(NCORES):
        for b in range(NB):
            rows = np.arange((c * NB + b) * 128, (c * NB + b) * 128 + 128)
            pos = offs[b]
            for r in rows:
                d = int(deg[r])
                lo = row_start[r]
                dsti[c, pos:pos + d] = dst[lo:lo + d]
                srcrel[c, pos:pos + d] = float(r & 127)
                ncw = int(degc[r])
                cbase = cell_offs[b] + (pos - offs[b]) // CELL
                cellsrc[c, cbase:cbase + ncw // CELL] = r
                pos += ncw
            assert pos <= offs[b + 1]

    # Per-cell s comes from on-chip PE selection: for each 128-cell span g of
    # block b, PM[:, g, :] is a one-hot [row-in-block i, cell-slot p] matrix
    # with PM[i, p] = 1 iff cell (b, g*128+p) belongs to row base_b + i.
    import ml_dtypes
    NMTOT = sum(n // 128 for n in ncellp)
    cores = []
    for c in range(NCORES):
        PMs = np.zeros((128, NMTOT, 128), ml_dtypes.bfloat16)
        g = 0
        for b in range(NB):
            base = (c * NB + b) * 128
            for m in range(ncellp[b] // 128):
                cs = cellsrc[c, cell_offs[b] + m * 128: cell_offs[b] + (m + 1) * 128]
                rel = cs.astype(np.int64) - base
                valid = (rel >= 0) & (rel < 128)
                PMs[rel[valid], g, np.arange(128)[valid]] = 1.0
                g += 1
        assert g == NMTOT
        cores.append({
            "dsti": dsti[c],
            "srcrel": srcrel[c].reshape(S // 128, 128).T.copy(),  # [128, S/128]
            "PMs": PMs,
        })
    return cores, [int(k) for k in Kb], S, [int(x) for x in ncellp]


# ---------------------------------------------------------------- device IR
def _build(Kb, S, ncellp):
    SC = sum(ncellp)
    NMTOT = SC // 128
    nc = bacc.Bacc("TRN2", target_bir_lowering=False, debug=False,
                   enable_asserts=False, num_devices=NCORES,
                   num_swdge_queues=4)
    XgT_d = nc.dram_tensor("XgT", [F, S], dt.bfloat16, kind="ExternalInput").ap()
    XTown_d = nc.dram_tensor("XTown", [F, R], dt.bfloat16, kind="ExternalInput").ap()
    Ws_d = nc.dram_tensor("Ws", [F, D], dt.float32, kind="ExternalInput").ap()
    WsT_d = nc.dram_tensor("WsT", [D, F], dt.float32, kind="ExternalInput").ap()
    apair_d = nc.dram_tensor("apair", [D, 2], dt.float32, kind="ExternalInput").ap()
    PMs_d = nc.dram_tensor("PMs", [128, NMTOT, 128], dt.bfloat16, kind="ExternalInput").ap()
    srel_d = nc.dram_tensor("srcrel", [128, S // 128], dt.float32, kind="ExternalInput").ap()
    sel16_d = nc.dram_tensor("sel16", [128, CELL], dt.float32, kind="ExternalInput").ap()
    E16_d = nc.dram_tensor("E16", [128, 128], dt.float32, kind="ExternalInput").ap()
    iotaf_d = nc.dram_tensor("iotaf", [128, 128], dt.float32, kind="ExternalInput").ap()
    out_d = nc.dram_tensor("out", [R, D], dt.float32, kind="ExternalOutput").ap()

    with tile.TileContext(nc) as tc, ExitStack() as ctx:
        cpool = ctx.enter_context(tc.tile_pool(name="const", bufs=1))
        xgpool = ctx.enter_context(tc.tile_pool(name="xg", bufs=2))
        wtpool = ctx.enter_context(tc.tile_pool(name="wt", bufs=2))
        ohpool = ctx.enter_context(tc.tile_pool(name="oh", bufs=2))
        Gpool = ctx.enter_context(tc.tile_pool(name="G", bufs=2))
        wpool = ctx.enter_context(tc.tile_pool(name="w", bufs=2))
        epool = ctx.enter_context(tc.tile_pool(name="ep", bufs=2))
        # PSUM budget (8 banks): wt 2x1, acc 2x1, sc 1, s-expand 2x1
        ps_wt = ctx.enter_context(tc.tile_pool(name="ps_wt", bufs=2, space="PSUM"))
        ps_acc = ctx.enter_context(tc.tile_pool(name="ps_acc", bufs=2, space="PSUM"))
        ps_sc = ctx.enter_context(tc.tile_pool(name="ps_sc", bufs=1, space="PSUM"))
        ps_se = ctx.enter_context(tc.tile_pool(name="ps_se", bufs=2, space="PSUM"))

        # ---- constants
        iota_f = cpool.tile([128, 128], dt.float32)
        nc.sync.dma_start(iota_f[:], iotaf_d)
        ws_t = cpool.tile([F, D], dt.float32)
        nc.sync.dma_start(ws_t[:], Ws_d)
        wsT_t = cpool.tile([D, F], dt.float32)
        nc.sync.dma_start(wsT_t[:], WsT_d)
        apair_t = cpool.tile([D, 2], dt.float32)
        nc.sync.dma_start(apair_t[:], apair_d)
        srel_t = cpool.tile([128, S // 128], dt.float32)
        nc.sync.dma_start(srel_t[:], srel_d)
        sel16_t = cpool.tile([128, CELL], dt.float32)
        nc.sync.dma_start(sel16_t[:], sel16_d)
        E16_t = cpool.tile([128, 128], dt.float32)
        nc.sync.dma_start(E16_t[:], E16_d)
        PMs_t = cpool.tile([128, NMTOT, 128], dt.bfloat16)
        nc.sync.dma_start(PMs_t[:], PMs_d)
        xtown_t = cpool.tile([F, NB, 128], dt.bfloat16)
        nc.scalar.dma_start(xtown_t[:], XTown_d.rearrange("f (b p) -> f b p", p=128))

        # Wse65 = [Ws@a2 | Ws] bf16: one 65-col rhs so each A1 matmul yields
        # [t_dst, Wh_dst] per slot directly.  Wsa1 bf16 for the s matmuls.
        wsa_ps = ps_sc.tile([128, 2], dt.float32, space="PSUM", tag="sc")
        nc.tensor.matmul(wsa_ps[:], lhsT=wsT_t[:], rhs=apair_t[:],
                         start=True, stop=True)
        Wse = cpool.tile([F, 1 + D], dt.bfloat16)
        nc.vector.tensor_copy(Wse[:, 0:1], wsa_ps[:, 0:1])
        nc.vector.tensor_copy(Wse[:, 1:1 + D], ws_t[:])
        wsa1_t = cpool.tile([F, 1], dt.bfloat16)
        nc.vector.tensor_copy(wsa1_t[:], wsa_ps[:, 1:2])

        # ---- s for own rows: s[r] = X[r] @ Ws @ a1, per block -> [128, NB]
        s_ps = ps_se.tile([128, NB], dt.float32, space="PSUM", tag="se")
        for b in range(NB):
            nc.tensor.matmul(s_ps[:, b:b + 1], lhsT=xtown_t[:, b, :],
                             rhs=wsa1_t[:], start=True, stop=True)
        # hi/lo bf16 split so the PM matmuls stay near-f32 exact
        sloc_hl = cpool.tile([128, NB, 2], dt.bfloat16)
        nc.vector.tensor_copy(sloc_hl[:, :, 0], s_ps[:])
        nc.vector.tensor_sub(sloc_hl[:, :, 1], s_ps[:], sloc_hl[:, :, 0])

        # ---- main loop: per 128-row block
        sl_offs = [0]
        for b in range(NB):
            sl_offs.append(sl_offs[-1] + Kb[b])
        outstage = cpool.tile([128, NB, D], dt.float32)
        gsp = 0          # global PM span index
        for b in range(NB):
            K = Kb[b]
            ncp = ncellp[b]                      # padded cell count (x128)
            nm = ncp // 128                      # 128-cell spans
            lo = sl_offs[b] * 128                # slot offset of this block
            # per-slot gathered X, feature-major: [128F, K, 128slot]
            xg = xgpool.tile([128, K, 128], dt.bfloat16)
            nc.scalar.dma_start(
                xg[:], XgT_d[:, lo:lo + K * 128]
                .rearrange("f (k p) -> f k p", p=128))
            # A1: per chunk [t, Wh] = XgT_c.T @ Wse  (slot-partition out),
            # sub-batched through one PSUM bank; t kept f32, Wh cast bf16
            t_f = wpool.tile([128, K], dt.float32, tag="tf")
            wh = wtpool.tile([128, K, D], dt.bfloat16)
            for c0 in range(0, K, SUB):
                c1 = min(c0 + SUB, K)
                wt_ps = ps_wt.tile([128, SUB, 1 + D], dt.float32,
                                   space="PSUM", tag="wt")
                for j in range(c1 - c0):
                    nc.tensor.matmul(wt_ps[:, j, :], lhsT=xg[:, c0 + j, :],
                                     rhs=Wse[:], start=True, stop=True)
                nc.vector.tensor_copy(t_f[:, c0:c1], wt_ps[:, 0:c1 - c0, 0])
                nc.vector.tensor_copy(wh[:, c0:c1, :], wt_ps[:, 0:c1 - c0, 1:])
            # one-hot of srcrel vs row-in-block (bf16)
            oht = ohpool.tile([128, K, 128], dt.bfloat16)
            nc.vector.tensor_tensor(
                out=oht[:],
                in0=iota_f[:, None, :].to_broadcast([128, K, 128]),
                in1=srel_t[:, sl_offs[b]:sl_offs[b] + K, None]
                    .to_broadcast([128, K, 128]),
                op=mybir.AluOpType.is_equal)
            # s per cell via PM one-hot matmuls, then expand to the edge
            # layout via constant matmuls (CPC = 128//CELL cells/chunk):
            #   s_edge[p, CELL*m+cl] = cellval[CPC*cl + p//CELL, m]
            sc_ps = ps_sc.tile([128, nm], dt.float32, space="PSUM", tag="sc")
            for m in range(nm):
                nc.tensor.matmul(sc_ps[:, m:m + 1], lhsT=PMs_t[:, gsp + m, :],
                                 rhs=sloc_hl[:, b, 0:1], start=True, stop=False)
                nc.tensor.matmul(sc_ps[:, m:m + 1], lhsT=PMs_t[:, gsp + m, :],
                                 rhs=sloc_hl[:, b, 1:2], start=False, stop=True)
            se_ps = ps_se.tile([128, nm * CELL], dt.float32, space="PSUM", tag="se")
            for m in range(nm):
                rhsm = wpool.tile([128, CELL], dt.float32, tag="rhsm")
                nc.vector.tensor_mul(
                    rhsm[:], sel16_t[:],
                    sc_ps[:, m:m + 1].to_broadcast([128, CELL]))
                nc.tensor.matmul(se_ps[:, m * CELL:(m + 1) * CELL], lhsT=E16_t[:],
                                 rhs=rhsm[:], start=True, stop=True)
            # w = exp(leaky(s + t))
            e_t = wpool.tile([128, K], dt.float32, tag="e")
            nc.vector.tensor_add(e_t[:], se_ps[:, 0:K], t_f[:])
            lk = wpool.tile([128, K], dt.float32, tag="lk")
            nc.vector.scalar_tensor_tensor(
                out=lk[:], in0=e_t[:], scalar=0.15, op0=mybir.AluOpType.mult,
                in1=e_t[:], op1=mybir.AluOpType.max)
            w_t = wpool.tile([128, K], dt.float32, tag="wt")
            nc.scalar.activation(w_t[:], lk[:], mybir.ActivationFunctionType.Exp)
            # G = [w * Wh_dst, w]  (bf16 for full-rate PE)
            G = Gpool.tile([128, K, D + 1], dt.bfloat16)
            nc.vector.tensor_mul(G[:, :, 0:D], wh[:],
                                 w_t[:, :, None].to_broadcast([128, K, D]))
            nc.vector.tensor_copy(G[:, :, D], w_t[:])
            # aggregate
            acc = ps_acc.tile([128, D + 1], dt.float32, space="PSUM", tag="acc")
            for c in range(K):
                nc.tensor.matmul(acc[:], lhsT=oht[:, c, :], rhs=G[:, c, :],
                                 start=(c == 0), stop=(c == K - 1))
            # epilogue: out = elu(U / Z)
            zg = epool.tile([128, 1], dt.float32, tag="zg")
            nc.vector.tensor_scalar_max(zg[:], acc[:, D:D + 1], 1e-30)
            zr = epool.tile([128, 1], dt.float32, tag="zr")
            nc.vector.reciprocal(zr[:], zg[:])
            x = epool.tile([128, D], dt.float32, tag="x")
            nc.vector.tensor_scalar_mul(x[:], acc[:, 0:D], zr[:])
            mn = epool.tile([128, D], dt.float32, tag="mn")
            nc.vector.tensor_scalar_min(mn[:], x[:], 0.0)
            em = epool.tile([128, D], dt.float32, tag="em")
            nc.scalar.activation(em[:], mn[:], mybir.ActivationFunctionType.Exp)
            rl = epool.tile([128, D], dt.float32, tag="rl")
            nc.vector.tensor_scalar_max(rl[:], x[:], 0.0)
            nc.vector.scalar_tensor_tensor(
                out=outstage[:, b, :], in0=em[:], scalar=-1.0,
                op0=mybir.AluOpType.add, in1=rl[:], op1=mybir.AluOpType.add)
            gsp += nm

        out_v = out_d.rearrange("(b p) d -> p b d", p=128)   # [128, NB, D]
        nc.sync.dma_start(out_v, outstage[:])
    nc.compile()
    return nc


_cache = {}


def _get_program(Kb, S, ncellp):
    key = (tuple(Kb), S, tuple(ncellp), tuple(sorted(_ABL)))
    if key not in _cache:
        _cache[key] = _build(Kb, S, ncellp)
    return _cache[key]


def make_in_maps(A, X, Ws, a):
    """Host-side sharding: returns (nc, in_maps)."""
    import ml_dtypes
    X = np.asarray(X, dtype=np.float32)
    Ws = np.ascontiguousarray(np.asarray(Ws, dtype=np.float32))
    a = np.asarray(a, dtype=np.float32).reshape(2 * D)
    Xbf = X.astype(ml_dtypes.bfloat16)
    WsT = np.ascontiguousarray(Ws.T)
    apair = np.stack([a[D:], a[:D]], axis=1).astype(np.float32)  # [D, 2] = [a2|a1]
    q = np.arange(128)
    CPC = 128 // CELL
    sel16 = (q[:, None] // CPC == np.arange(CELL)[None, :]).astype(np.float32)
    E16 = (q[:, None] % CPC == q[None, :] // CELL).astype(np.float32)
    iotaf = np.tile(np.arange(128, dtype=np.float32)[None, :], (128, 1))
    cores, Kb, S, ncellp = _prep_edges(A)
    nc = _get_program(Kb, S, ncellp)
    in_maps = []
    for c in range(NCORES):
        ci = cores[c]
        XgT = np.ascontiguousarray(Xbf[ci["dsti"]].T)          # [F, S]
        XTown = np.ascontiguousarray(Xbf[c * R:(c + 1) * R].T)  # [F, R]
        in_maps.append({
            "XgT": XgT, "XTown": XTown, "Ws": Ws, "WsT": WsT,
            "apair": apair, "sel16": sel16, "E16": E16, "iotaf": iotaf,
            "srcrel": ci["srcrel"], "PMs": ci["PMs"],
        })
    return nc, in_maps


def kernel(A, X, Ws, a):
    nc, in_maps = make_in_maps(A, X, Ws, a)
    res = run_bass_kernel_spmd(nc, in_maps, core_ids=list(range(NCORES)),
                               trace=False)
    return np.concatenate([r["out"] for r in res.results], axis=0)


# revision 11
# speedup vs baseline: 3.9113x; 1.4434x over previous
"""GAT forward kernel for Trainium2 (8 NeuronCores, Bass/Tile).

Reference computation (dense form):
    adj = densify(A); Wh = X @ Ws; e = leaky_relu(Wh@a1 + (Wh@a2).T, 0.15)
    att = softmax(where(adj>0, e, -9e15), axis=1); out = elu(att @ Wh)

Sparse form (only ~524K of 16384^2 entries matter; |e| <= ~16 so softmax
needs no max-subtraction):
    w_e = exp(leaky(s_src + t_dst));  out_i = elu(sum_e w_e Wh_dst / sum_e w_e)

Sharding: rows (softmax queries) split 2048/core across 8 cores.

Two-tier edge layout, host pre-gathers X[dst_e] per slot (contiguous bf16
DMA, no device gather):

Tier 1 (first Q=32 edges of every row): slot (b, q, p) holds edge q of row
(b, p).  One matmul per (b, q) chunk (lhsT = XgT chunk, rhs = [Ws@a2|Ws|1])
gives [t_dst, Wh_dst, 1] for 128 rows at once, ROW-partition.  s_src is a
per-partition scalar broadcast, pad slots are masked with -1e30 before exp,
and the softmax aggregation is a plain DVE reduction over q -- no one-hot
matmuls, no PM machinery.

Tier 2 (edges Q..deg of rows with deg > Q, ~7% of slots): classic row-major
cell layout with per-chunk one-hot (is_equal vs srcrel) aggregation matmuls
and the PM cell trick for per-slot s.  Its acc joins tier 1's U in the
epilogue: out = elu((U1+U2) / (Z1+Z2)).

Host prep pads tiers to cross-core uniform chunk counts so all 8 cores run
the same program (SPMD).
"""
import os
import sys

if "/opt/trn_rl_repo" not in sys.path:
    sys.path.insert(0, "/opt/trn_rl_repo")

_ABL = set(os.environ.get("GAT_ABLATE", "").split(","))

from contextlib import ExitStack

import numpy as np

import concourse.bass as bass
import concourse.tile as tile
from concourse import bacc, mybir
from concourse.bass_utils import run_bass_kernel_spmd

N = 16384          # nodes
F = 128            # input features
D = 64             # embedding dim
NCORES = 8
R = N // NCORES    # rows per core (2048)
NB = R // 128      # row blocks per core (16)
Q = 32             # tier-1 slots per row
S1 = NB * Q * 128  # tier-1 slots per core (65536)
CELL = 8           # tier-2 slots per cell (one src row per cell)
SUB = 7            # A1 chunks per PSUM sub-batch (7*66*4B < 2KB bank)
NEG = -1.0e30
dt = mybir.dt


# ---------------------------------------------------------------- host prep
def _prep_edges(A):
    """Dedup edges; tier-1 = first Q edges per row in (block, q, row) slot
    order with a -inf pad mask; tier-2 = remaining edges in row-major
    CELL-padded layout, padded to cross-core uniform chunk counts Kb2."""
    src_all = np.asarray(A[0], dtype=np.int64)
    dst_all = np.asarray(A[1], dtype=np.int64)
    keys = np.unique(src_all * N + dst_all)     # dedup + sort by (src, dst)
    src = (keys // N).astype(np.int32)
    dst = (keys % N).astype(np.int32)
    E = len(dst)

    deg = np.bincount(src, minlength=N)
    assert deg.min() > 0, (
        "empty rows present; this kernel assumes every row has >=1 edge"
    )
    row_start = np.concatenate([[0], np.cumsum(deg)])

    # ---- tier 1: dsti1 [NCORES, NB, Q, 128], maskneg [NCORES, 128, NB, Q]
    rows = np.arange(N).reshape(NCORES, NB, 128)
    qs = np.arange(Q)
    pos = row_start[rows][..., None] + qs                 # [C, NB, 128, Q]
    valid = qs[None, None, None, :] < deg[rows][..., None]
    dsti1 = np.where(valid, dst[np.minimum(pos, E - 1)], 0)
    dsti1 = dsti1.transpose(0, 1, 3, 2).astype(np.int32)  # [C, NB, Q, 128]
    maskneg = np.where(valid, np.float32(0), np.float32(NEG))
    maskneg = maskneg.transpose(0, 2, 1, 3).copy()        # [C, 128, NB, Q]

    # ---- tier 2: excess edges, row-major cells, block buckets
    exc = np.maximum(deg - Q, 0)
    excc = ((exc + CELL - 1) // CELL) * CELL
    slots_cb = excc.reshape(NCORES, NB, 128).sum(axis=2)   # [C, NB]
    Kb2 = np.maximum((slots_cb.max(axis=0) + 127) // 128, 1)   # [NB]
    S2 = int(Kb2.sum()) * 128
    offs2 = np.concatenate([[0], np.cumsum(Kb2)]) * 128
    cells_cb = slots_cb // CELL
    ncell2 = [int(k) * (128 // CELL) for k in Kb2]
    ncellp2 = [((n + 127) // 128) * 128 for n in ncell2]
    cell_offs2 = np.concatenate([[0], np.cumsum(ncellp2)])
    SC2 = int(cell_offs2[-1])

    dsti2 = np.zeros((NCORES, S2), np.int32)
    srel2 = np.full((NCORES, S2), -1.0, np.float32)
    cellsrc2 = np.zeros((NCORES, SC2), np.int16)
    for c in range(NCORES):
        for b in range(NB):
            pos2 = offs2[b]
            for p in range(128):
                r = (c * NB + b) * 128 + p
                d = int(exc[r])
                if d == 0:
                    continue
                lo = row_start[r] + Q
                dsti2[c, pos2:pos2 + d] = dst[lo:lo + d]
                srel2[c, pos2:pos2 + d] = float(p)
                ncw = int(excc[r])
                cbase = cell_offs2[b] + (pos2 - offs2[b]) // CELL
                cellsrc2[c, cbase:cbase + ncw // CELL] = r
                pos2 += ncw
            assert pos2 <= offs2[b + 1]

    import ml_dtypes
    NMTOT = sum(n // 128 for n in ncellp2)
    cores = []
    for c in range(NCORES):
        PMs = np.zeros((128, NMTOT, 128), ml_dtypes.bfloat16)
        g = 0
        for b in range(NB):
            base = (c * NB + b) * 128
            for m in range(ncellp2[b] // 128):
                cs = cellsrc2[c, cell_offs2[b] + m * 128:
                              cell_offs2[b] + (m + 1) * 128]
                rel = cs.astype(np.int64) - base
                vv = (rel >= 0) & (rel < 128)
                PMs[rel[vv], g, np.arange(128)[vv]] = 1.0
                g += 1
        assert g == NMTOT
        cores.append({
            "dsti": np.concatenate([dsti1[c].reshape(-1), dsti2[c]]),
            "maskneg": maskneg[c],                               # [128, NB, Q]
            "srel2": srel2[c].reshape(S2 // 128, 128).T.copy(),  # [128, S2/128]
            "PMs": PMs,
        })
    return cores, [int(k) for k in Kb2], S2, [int(x) for x in ncellp2]


# ---------------------------------------------------------------- device IR
def _build(Kb2, S2, ncellp2):
    SC2 = sum(ncellp2)
    NMTOT = SC2 // 128
    ST = S1 + S2
    nc = bacc.Bacc("TRN2", target_bir_lowering=False, debug=False,
                   enable_asserts=False, num_devices=NCORES,
                   num_swdge_queues=4)
    XgT_d = nc.dram_tensor("XgT", [F, ST], dt.bfloat16, kind="ExternalInput").ap()
    XTown_d = nc.dram_tensor("XTown", [F, R], dt.bfloat16, kind="ExternalInput").ap()
    Ws_d = nc.dram_tensor("Ws", [F, D], dt.float32, kind="ExternalInput").ap()
    WsT_d = nc.dram_tensor("WsT", [D, F], dt.float32, kind="ExternalInput").ap()
    apair_d = nc.dram_tensor("apair", [D, 2], dt.float32, kind="ExternalInput").ap()
    mask_d = nc.dram_tensor("maskneg", [128, NB, Q], dt.float32, kind="ExternalInput").ap()
    PMs_d = nc.dram_tensor("PMs", [128, NMTOT, 128], dt.bfloat16, kind="ExternalInput").ap()
    srel2_d = nc.dram_tensor("srel2", [128, S2 // 128], dt.float32, kind="ExternalInput").ap()
    sel16_d = nc.dram_tensor("sel16", [128, CELL], dt.float32, kind="ExternalInput").ap()
    E16_d = nc.dram_tensor("E16", [128, 128], dt.float32, kind="ExternalInput").ap()
    iotaf_d = nc.dram_tensor("iotaf", [128, 128], dt.float32, kind="ExternalInput").ap()
    out_d = nc.dram_tensor("out", [R, D], dt.float32, kind="ExternalOutput").ap()
    if "dbg" in _ABL:
        dbg_d = nc.dram_tensor("dbg", [128, 4 * Q + 2 * (1 + D)], dt.float32,
                               kind="ExternalOutput").ap()

    with tile.TileContext(nc) as tc, ExitStack() as ctx:
        cpool = ctx.enter_context(tc.tile_pool(name="const", bufs=1))
        xgpool = ctx.enter_context(tc.tile_pool(name="xg", bufs=2))
        x2pool = ctx.enter_context(tc.tile_pool(name="x2", bufs=2))
        whpool = ctx.enter_context(tc.tile_pool(name="wh", bufs=2))
        Gpool = ctx.enter_context(tc.tile_pool(name="G", bufs=2))
        ohpool = ctx.enter_context(tc.tile_pool(name="oh", bufs=2))
        wpool = ctx.enter_context(tc.tile_pool(name="w", bufs=2))
        epool = ctx.enter_context(tc.tile_pool(name="ep", bufs=2))
        # PSUM budget (8 banks): wt 2x1, acc2 2x1, sc 1, se 2x1
        ps_wt = ctx.enter_context(tc.tile_pool(name="ps_wt", bufs=2, space="PSUM"))
        ps_acc = ctx.enter_context(tc.tile_pool(name="ps_acc", bufs=2, space="PSUM"))
        ps_sc = ctx.enter_context(tc.tile_pool(name="ps_sc", bufs=1, space="PSUM"))
        ps_se = ctx.enter_context(tc.tile_pool(name="ps_se", bufs=2, space="PSUM"))

        # ---- constants
        iota_f = cpool.tile([128, 128], dt.float32)
        nc.sync.dma_start(iota_f[:], iotaf_d)
        ws_t = cpool.tile([F, D], dt.float32)
        nc.sync.dma_start(ws_t[:], Ws_d)
        wsT_t = cpool.tile([D, F], dt.float32)
        nc.sync.dma_start(wsT_t[:], WsT_d)
        apair_t = cpool.tile([D, 2], dt.float32)
        nc.sync.dma_start(apair_t[:], apair_d)
        mask_t = cpool.tile([128, NB, Q], dt.float32)
        nc.sync.dma_start(mask_t[:], mask_d)
        srel2_t = cpool.tile([128, S2 // 128], dt.float32)
        nc.sync.dma_start(srel2_t[:], srel2_d)
        sel16_t = cpool.tile([128, CELL], dt.float32)
        nc.sync.dma_start(sel16_t[:], sel16_d)
        E16_t = cpool.tile([128, 128], dt.float32)
        nc.sync.dma_start(E16_t[:], E16_d)
        PMs_t = cpool.tile([128, NMTOT, 128], dt.bfloat16)
        nc.sync.dma_start(PMs_t[:], PMs_d)
        xtown_t = cpool.tile([F, NB, 128], dt.bfloat16)
        nc.scalar.dma_start(xtown_t[:], XTown_d.rearrange("f (b p) -> f b p", p=128))

        # Wse = [Ws@a2 | Ws] bf16: one 65-col rhs so each A1 matmul yields
        # [t_dst, Wh_dst] per slot.  Wsa1 bf16 for the s matmuls.
        wsa_ps = ps_sc.tile([128, 2], dt.float32, space="PSUM", tag="sc")
        nc.tensor.matmul(wsa_ps[:], lhsT=wsT_t[:], rhs=apair_t[:],
                         start=True, stop=True)
        Wse = cpool.tile([F, 1 + D], dt.bfloat16)
        nc.vector.tensor_copy(Wse[:, 0:1], wsa_ps[:, 0:1])
        nc.vector.tensor_copy(Wse[:, 1:1 + D], ws_t[:])
        wsa1_t = cpool.tile([F, 1], dt.bfloat16)
        nc.vector.tensor_copy(wsa1_t[:], wsa_ps[:, 1:2])

        # ---- s for own rows: s[r] = X[r] @ Ws @ a1, per block -> [128, NB]
        s_ps = ps_se.tile([128, NB], dt.float32, space="PSUM", tag="se")
        for b in range(NB):
            nc.tensor.matmul(s_ps[:, b:b + 1], lhsT=xtown_t[:, b, :],
                             rhs=wsa1_t[:], start=True, stop=True)
        sloc = cpool.tile([128, NB], dt.float32)
        nc.vector.tensor_copy(sloc[:], s_ps[:])
        # hi/lo bf16 split so the tier-2 PM matmuls stay near-f32 exact
        sloc_hl = cpool.tile([128, NB, 2], dt.bfloat16)
        nc.vector.tensor_copy(sloc_hl[:, :, 0], s_ps[:])
        nc.vector.tensor_sub(sloc_hl[:, :, 1], s_ps[:], sloc_hl[:, :, 0])

        # ---- main loop: per 128-row block
        sl2 = [0]
        for b in range(NB):
            sl2.append(sl2[-1] + Kb2[b])
        XgT1_v = XgT_d[:, 0:S1].rearrange("f (b q p) -> f b q p", q=Q, p=128)
        outstage = cpool.tile([128, NB, D], dt.float32)
        gsp = 0          # global PM span index
        for b in range(NB):
            K2 = Kb2[b]
            nm2 = ncellp2[b] // 128
            # ================= tier 1 =================
            xg1 = xgpool.tile([128, Q, 128], dt.bfloat16)
            nc.scalar.dma_start(xg1[:], XgT1_v[:, b])
            t_f = wpool.tile([128, Q], dt.float32, tag="tf")
            whp = whpool.tile([128, 1 + D, Q], dt.bfloat16)   # [Wh|1][d, q]
            nc.vector.memset(whp[:, D, :], 1.0)
            for c0 in range(0, Q, SUB):
                c1 = min(c0 + SUB, Q)
                wt_ps = ps_wt.tile([128, SUB, 1 + D], dt.float32,
                                   space="PSUM", tag="wt")
                for j in range(c1 - c0):
                    nc.tensor.matmul(wt_ps[:, j, :], lhsT=xg1[:, c0 + j, :],
                                     rhs=Wse[:], start=True, stop=True)
                nc.vector.tensor_copy(t_f[:, c0:c1], wt_ps[:, 0:c1 - c0, 0])
                nc.scalar.activation(
                    whp[:, 0:D, c0:c1],
                    wt_ps[:, 0:c1 - c0, 1:].rearrange("p q d -> p d q"),
                    mybir.ActivationFunctionType.Copy)
            # w = exp(leaky(s + t) + mask)
            e_t = wpool.tile([128, Q], dt.float32, tag="e")
            nc.vector.tensor_scalar_add(e_t[:], t_f[:], sloc[:, b:b + 1])
            lk = wpool.tile([128, Q], dt.float32, tag="lk")
            nc.vector.scalar_tensor_tensor(
                out=lk[:], in0=e_t[:], scalar=0.15, op0=mybir.AluOpType.mult,
                in1=e_t[:], op1=mybir.AluOpType.max)
            lkm = wpool.tile([128, Q], dt.float32, tag="lkm")
            nc.vector.tensor_add(lkm[:], lk[:], mask_t[:, b, :])
            w_t = wpool.tile([128, Q], dt.float32, tag="wt")
            nc.scalar.activation(w_t[:], lkm[:], mybir.ActivationFunctionType.Exp)
            # G[d, q] = w[q] * [Wh, 1][d, q]; U[d] = sum_q G[d, q]
            G = Gpool.tile([128, 1 + D, Q], dt.bfloat16)
            nc.vector.tensor_mul(G[:], whp[:],
                                 w_t[:, None, :].to_broadcast([128, 1 + D, Q]))
            U_t = epool.tile([128, 1 + D], dt.float32, tag="U")
            nc.vector.reduce_sum(U_t[:], G[:], axis=mybir.AxisListType.X)
            if "dbg" in _ABL and b == 0:
                dstage = cpool.tile([128, 4 * Q + 2 * (1 + D)], dt.float32)
                nc.vector.tensor_copy(dstage[:, 0:Q], t_f[:])
                nc.vector.tensor_copy(dstage[:, Q:2 * Q], e_t[:])
                nc.vector.tensor_copy(dstage[:, 2 * Q:3 * Q], lkm[:])
                nc.vector.tensor_copy(dstage[:, 3 * Q:4 * Q], w_t[:])
                nc.vector.tensor_copy(dstage[:, 4 * Q:4 * Q + 1 + D], U_t[:])
                nc.vector.tensor_copy(
                    dstage[:, 4 * Q + 1 + D:4 * Q + 2 * (1 + D)],
                    G[:, :, 0])
                nc.sync.dma_start(dbg_d, dstage[:])
            # ================= tier 2 =================
            lo2 = S1 + sl2[b] * 128
            xg2 = x2pool.tile([128, K2, 128], dt.bfloat16)
            nc.scalar.dma_start(
                xg2[:], XgT_d[:, lo2:lo2 + K2 * 128]
                .rearrange("f (k p) -> f k p", p=128))
            t2_f = wpool.tile([128, K2], dt.float32, tag="t2")
            wh2 = whpool.tile([128, K2, 1 + D], dt.bfloat16)
            for c0 in range(0, K2, SUB):
                c1 = min(c0 + SUB, K2)
                wt_ps = ps_wt.tile([128, SUB, 1 + D], dt.float32,
                                   space="PSUM", tag="wt")
                for j in range(c1 - c0):
                    nc.tensor.matmul(wt_ps[:, j, :], lhsT=xg2[:, c0 + j, :],
                                     rhs=Wse[:], start=True, stop=True)
                nc.vector.tensor_copy(t2_f[:, c0:c1], wt_ps[:, 0:c1 - c0, 0])
                nc.scalar.activation(
                    wh2[:, c0:c1, 0:D], wt_ps[:, 0:c1 - c0, 1:],
                    mybir.ActivationFunctionType.Copy)
            # one-hot of srcrel vs row-in-block
            oht2 = ohpool.tile([128, K2, 128], dt.bfloat16)
            nc.vector.tensor_tensor(
                out=oht2[:],
                in0=iota_f[:, None, :].to_broadcast([128, K2, 128]),
                in1=srel2_t[:, sl2[b]:sl2[b] + K2, None]
                    .to_broadcast([128, K2, 128]),
                op=mybir.AluOpType.is_equal)
            # s per cell via PM one-hot matmuls, then expand cells -> slots
            sc_ps = ps_sc.tile([128, nm2], dt.float32, space="PSUM", tag="sc")
            for m in range(nm2):
                nc.tensor.matmul(sc_ps[:, m:m + 1], lhsT=PMs_t[:, gsp + m, :],
                                 rhs=sloc_hl[:, b, 0:1], start=True, stop=False)
                nc.tensor.matmul(sc_ps[:, m:m + 1], lhsT=PMs_t[:, gsp + m, :],
                                 rhs=sloc_hl[:, b, 1:2], start=False, stop=True)
            se_ps = ps_se.tile([128, nm2 * CELL], dt.float32, space="PSUM", tag="se")
            for m in range(nm2):
                rhsm = wpool.tile([128, CELL], dt.float32, tag="rhsm")
                nc.vector.tensor_mul(
                    rhsm[:], sel16_t[:],
                    sc_ps[:, m:m + 1].to_broadcast([128, CELL]))
                nc.tensor.matmul(se_ps[:, m * CELL:(m + 1) * CELL],
                                 lhsT=E16_t[:], rhs=rhsm[:],
                                 start=True, stop=True)
            e2 = wpool.tile([128, K2], dt.float32, tag="e2")
            nc.vector.tensor_add(e2[:], se_ps[:, 0:K2], t2_f[:])
            lk2 = wpool.tile([128, K2], dt.float32, tag="lk2")
            nc.vector.scalar_tensor_tensor(
                out=lk2[:], in0=e2[:], scalar=0.15, op0=mybir.AluOpType.mult,
                in1=e2[:], op1=mybir.AluOpType.max)
            w2 = wpool.tile([128, K2], dt.float32, tag="w2")
            nc.scalar.activation(w2[:], lk2[:], mybir.ActivationFunctionType.Exp)
            G2 = Gpool.tile([128, K2, 1 + D], dt.bfloat16)
            nc.vector.tensor_mul(G2[:, :, 0:D], wh2[:, :, 0:D],
                                 w2[:, :, None].to_broadcast([128, K2, D]))
            nc.vector.tensor_copy(G2[:, :, D], w2[:])
            acc2 = ps_acc.tile([128, 1 + D], dt.float32, space="PSUM", tag="acc")
            for c in range(K2):
                nc.tensor.matmul(acc2[:], lhsT=oht2[:, c, :], rhs=G2[:, c, :],
                                 start=(c == 0), stop=(c == K2 - 1))
            # ================= epilogue: out = elu((U1+U2)/(Z1+Z2)) =========
            Ut = epool.tile([128, 1 + D], dt.float32, tag="Ut")
            nc.vector.tensor_add(Ut[:], U_t[:], acc2[:])
            zg = epool.tile([128, 1], dt.float32, tag="zg")
            nc.vector.tensor_scalar_max(zg[:], Ut[:, D:D + 1], 1e-30)
            zr = epool.tile([128, 1], dt.float32, tag="zr")
            nc.vector.reciprocal(zr[:], zg[:])
            x = epool.tile([128, D], dt.float32, tag="x")
            nc.vector.tensor_scalar_mul(x[:], Ut[:, 0:D], zr[:])
            mn = epool.tile([128, D], dt.float32, tag="mn")
            nc.vector.tensor_scalar_min(mn[:], x[:], 0.0)
            em = epool.tile([128, D], dt.float32, tag="em")
            nc.scalar.activation(em[:], mn[:], mybir.ActivationFunctionType.Exp)
            rl = epool.tile([128, D], dt.float32, tag="rl")
            nc.vector.tensor_scalar_max(rl[:], x[:], 0.0)
            nc.vector.scalar_tensor_tensor(
                out=outstage[:, b, :], in0=em[:], scalar=-1.0,
                op0=mybir.AluOpType.add, in1=rl[:], op1=mybir.AluOpType.add)
            gsp += nm2

        out_v = out_d.rearrange("(b p) d -> p b d", p=128)   # [128, NB, D]
        nc.sync.dma_start(out_v, outstage[:])
    nc.compile()
    return nc


_cache = {}


def _get_program(Kb2, S2, ncellp2):
    key = (tuple(Kb2), S2, tuple(ncellp2), tuple(sorted(_ABL)))
    if key not in _cache:
        _cache[key] = _build(Kb2, S2, ncellp2)
    return _cache[key]


def make_in_maps(A, X, Ws, a):
    """Host-side sharding: returns (nc, in_maps)."""
    import ml_dtypes
    X = np.asarray(X, dtype=np.float32)
    Ws = np.ascontiguousarray(np.asarray(Ws, dtype=np.float32))
    a = np.asarray(a, dtype=np.float32).reshape(2 * D)
    Xbf = X.astype(ml_dtypes.bfloat16)
    WsT = np.ascontiguousarray(Ws.T)
    apair = np.stack([a[D:], a[:D]], axis=1).astype(np.float32)  # [D, 2] = [a2|a1]
    q = np.arange(128)
    CPC = 128 // CELL
    sel16 = (q[:, None] // CPC == np.arange(CELL)[None, :]).astype(np.float32)
    E16 = (q[:, None] % CPC == q[None, :] // CELL).astype(np.float32)
    iotaf = np.tile(np.arange(128, dtype=np.float32)[None, :], (128, 1))
    cores, Kb2, S2, ncellp2 = _prep_edges(A)
    nc = _get_program(Kb2, S2, ncellp2)
    in_maps = []
    for c in range(NCORES):
        ci = cores[c]
        XgT = np.ascontiguousarray(Xbf[ci["dsti"]].T)          # [F, S1+S2]
        XTown = np.ascontiguousarray(Xbf[c * R:(c + 1) * R].T)  # [F, R]
        in_maps.append({
            "XgT": XgT, "XTown": XTown, "Ws": Ws, "WsT": WsT,
            "apair": apair, "maskneg": ci["maskneg"],
            "sel16": sel16, "E16": E16, "iotaf": iotaf,
            "srel2": ci["srel2"], "PMs": ci["PMs"],
        })
    return nc, in_maps


def kernel(A, X, Ws, a):
    nc, in_maps = make_in_maps(A, X, Ws, a)
    res = run_bass_kernel_spmd(nc, in_maps, core_ids=list(range(NCORES)),
                               trace=False)
    return np.concatenate([r["out"] for r in res.results], axis=0)


# revision 18
# speedup vs baseline: 4.6927x; 1.1998x over previous
"""GAT forward kernel for Trainium2 (8 NeuronCores, Bass/Tile).

Reference computation (dense form):
    adj = densify(A); Wh = X @ Ws; e = leaky_relu(Wh@a1 + (Wh@a2).T, 0.15)
    att = softmax(where(adj>0, e, -9e15), axis=1); out = elu(att @ Wh)

Sparse form (only ~524K of 16384^2 entries matter; |e| <= ~16 so softmax
needs no max-subtraction):
    w_e = exp(leaky(s_src + t_dst));  out_i = elu(sum_e w_e Wh_dst / sum_e w_e)

Sharding: rows (softmax queries) split 2048/core across 8 cores.

Two-tier edge layout, host pre-gathers X[dst_e] per slot (contiguous bf16
DMA, no device gather):

Tier 1 (first Q=32 edges of every row): slot (b, q, p) holds edge q of row
(b, p).  One matmul per (b, q) chunk (lhsT = XgT chunk, rhs = [Ws@a2|Ws|1])
gives [t_dst, Wh_dst, 1] for 128 rows at once, ROW-partition.  s_src is a
per-partition scalar broadcast, pad slots are masked with -1e30 before exp,
and the softmax aggregation is a plain DVE reduction over q -- no one-hot
matmuls, no PM machinery.

Tier 2 (edges Q..deg of rows with deg > Q, ~7% of slots): classic row-major
cell layout with per-chunk one-hot (is_equal vs srcrel) aggregation matmuls
and the PM cell trick for per-slot s.  Its acc joins tier 1's U in the
epilogue: out = elu((U1+U2) / (Z1+Z2)).

Host prep pads tiers to cross-core uniform chunk counts so all 8 cores run
the same program (SPMD).
"""
import os
import sys

if "/opt/trn_rl_repo" not in sys.path:
    sys.path.insert(0, "/opt/trn_rl_repo")

_ABL = set(os.environ.get("GAT_ABLATE", "").split(","))

from contextlib import ExitStack

import numpy as np

import concourse.bass as bass
import concourse.tile as tile
from concourse import bacc, mybir
from concourse.bass_utils import run_bass_kernel_spmd

N = 16384          # nodes
F = 128            # input features
D = 64             # embedding dim
NCORES = 8
R = N // NCORES    # rows per core (2048)
NB = R // 128      # row blocks per core (16)
Q = 32             # tier-1 slots per row
S1 = NB * Q * 128  # tier-1 slots per core (65536)
CELL = 8           # tier-2 slots per cell (one src row per cell)
SUB = 7            # A1 chunks per PSUM sub-batch (7*66*4B < 2KB bank)
NEG = -1.0e30
dt = mybir.dt


# ---------------------------------------------------------------- host prep
def _prep_edges(A):
    """Dedup edges; tier-1 = first Q edges per row in (block, q, row) slot
    order with a -inf pad mask; tier-2 = remaining edges in row-major
    CELL-padded layout, padded to cross-core uniform chunk counts Kb2."""
    src_all = np.asarray(A[0], dtype=np.int64)
    dst_all = np.asarray(A[1], dtype=np.int64)
    keys = np.unique(src_all * N + dst_all)     # dedup + sort by (src, dst)
    src = (keys // N).astype(np.int32)
    dst = (keys % N).astype(np.int32)
    E = len(dst)

    deg = np.bincount(src, minlength=N)
    assert deg.min() > 0, (
        "empty rows present; this kernel assumes every row has >=1 edge"
    )
    row_start = np.concatenate([[0], np.cumsum(deg)])

    # ---- tier 1: dsti1 [NCORES, NB, Q, 128], maskneg [NCORES, 128, NB, Q]
    rows = np.arange(N).reshape(NCORES, NB, 128)
    qs = np.arange(Q)
    pos = row_start[rows][..., None] + qs                 # [C, NB, 128, Q]
    valid = qs[None, None, None, :] < deg[rows][..., None]
    dsti1 = np.where(valid, dst[np.minimum(pos, E - 1)], 0)
    dsti1 = dsti1.transpose(0, 1, 3, 2).astype(np.int32)  # [C, NB, Q, 128]
    maskneg = np.where(valid, np.float32(0), np.float32(NEG))
    maskneg = maskneg.transpose(0, 2, 1, 3).copy()        # [C, 128, NB, Q]

    # ---- tier 2: excess edges, row-major cells, block buckets
    exc = np.maximum(deg - Q, 0)
    excc = ((exc + CELL - 1) // CELL) * CELL
    slots_cb = excc.reshape(NCORES, NB, 128).sum(axis=2)   # [C, NB]
    Kb2 = np.maximum((slots_cb.max(axis=0) + 127) // 128, 1)   # [NB]
    S2 = int(Kb2.sum()) * 128
    offs2 = np.concatenate([[0], np.cumsum(Kb2)]) * 128
    cells_cb = slots_cb // CELL
    ncell2 = [int(k) * (128 // CELL) for k in Kb2]
    ncellp2 = [((n + 127) // 128) * 128 for n in ncell2]
    cell_offs2 = np.concatenate([[0], np.cumsum(ncellp2)])
    SC2 = int(cell_offs2[-1])

    dsti2 = np.zeros((NCORES, S2), np.int32)
    srel2 = np.full((NCORES, S2), -1.0, np.float32)
    cellsrc2 = np.zeros((NCORES, SC2), np.int16)
    for c in range(NCORES):
        for b in range(NB):
            pos2 = offs2[b]
            for p in range(128):
                r = (c * NB + b) * 128 + p
                d = int(exc[r])
                if d == 0:
                    continue
                lo = row_start[r] + Q
                dsti2[c, pos2:pos2 + d] = dst[lo:lo + d]
                srel2[c, pos2:pos2 + d] = float(p)
                ncw = int(excc[r])
                cbase = cell_offs2[b] + (pos2 - offs2[b]) // CELL
                cellsrc2[c, cbase:cbase + ncw // CELL] = r
                pos2 += ncw
            assert pos2 <= offs2[b + 1]

    import ml_dtypes
    NMTOT = sum(n // 128 for n in ncellp2)
    cores = []
    for c in range(NCORES):
        PMs = np.zeros((128, NMTOT, 128), ml_dtypes.bfloat16)
        g = 0
        for b in range(NB):
            base = (c * NB + b) * 128
            for m in range(ncellp2[b] // 128):
                cs = cellsrc2[c, cell_offs2[b] + m * 128:
                              cell_offs2[b] + (m + 1) * 128]
                rel = cs.astype(np.int64) - base
                vv = (rel >= 0) & (rel < 128)
                PMs[rel[vv], g, np.arange(128)[vv]] = 1.0
                g += 1
        assert g == NMTOT
        cores.append({
            "dsti": np.concatenate([dsti1[c].reshape(-1), dsti2[c]]),
            "maskneg": maskneg[c],                               # [128, NB, Q]
            "srel2": srel2[c].reshape(S2 // 128, 128).T.copy(),  # [128, S2/128]
            "PMs": PMs,
        })
    return cores, [int(k) for k in Kb2], S2, [int(x) for x in ncellp2]


# ---------------------------------------------------------------- device IR
def _build(Kb2, S2, ncellp2):
    SC2 = sum(ncellp2)
    NMTOT = SC2 // 128
    ST = S1 + S2
    nc = bacc.Bacc("TRN2", target_bir_lowering=False, debug=False,
                   enable_asserts=False, num_devices=NCORES,
                   num_swdge_queues=4)
    XgT_d = nc.dram_tensor("XgT", [F, ST], dt.bfloat16, kind="ExternalInput").ap()
    XTown_d = nc.dram_tensor("XTown", [F, R], dt.bfloat16, kind="ExternalInput").ap()
    Ws_d = nc.dram_tensor("Ws", [F, D], dt.float32, kind="ExternalInput").ap()
    WsT_d = nc.dram_tensor("WsT", [D, F], dt.float32, kind="ExternalInput").ap()
    apair_d = nc.dram_tensor("apair", [D, 2], dt.float32, kind="ExternalInput").ap()
    mask_d = nc.dram_tensor("maskneg", [128, NB, Q], dt.float32, kind="ExternalInput").ap()
    PMs_d = nc.dram_tensor("PMs", [128, NMTOT, 128], dt.bfloat16, kind="ExternalInput").ap()
    srel2_d = nc.dram_tensor("srel2", [128, S2 // 128], dt.float32, kind="ExternalInput").ap()
    sel16_d = nc.dram_tensor("sel16", [128, CELL], dt.float32, kind="ExternalInput").ap()
    E16_d = nc.dram_tensor("E16", [128, 128], dt.float32, kind="ExternalInput").ap()
    iotaf_d = nc.dram_tensor("iotaf", [128, 128], dt.float32, kind="ExternalInput").ap()
    out_d = nc.dram_tensor("out", [R, D], dt.float32, kind="ExternalOutput").ap()
    if "dbg" in _ABL:
        dbg_d = nc.dram_tensor("dbg", [128, 4 * Q + 2 * (1 + D)], dt.float32,
                               kind="ExternalOutput").ap()

    with tile.TileContext(nc) as tc, ExitStack() as ctx:
        cpool = ctx.enter_context(tc.tile_pool(name="const", bufs=1))
        xgpool = ctx.enter_context(tc.tile_pool(name="xg", bufs=2))
        x2pool = ctx.enter_context(tc.tile_pool(name="x2", bufs=2))
        whpool = ctx.enter_context(tc.tile_pool(name="wh", bufs=2))
        Gpool = ctx.enter_context(tc.tile_pool(name="G", bufs=2))
        ohpool = ctx.enter_context(tc.tile_pool(name="oh", bufs=2))
        wpool = ctx.enter_context(tc.tile_pool(name="w", bufs=2))
        epool = ctx.enter_context(tc.tile_pool(name="ep", bufs=2))
        # PSUM budget (8 banks): wt 2x1, acc2 2x1, sc 1, se 2x1
        ps_wt = ctx.enter_context(tc.tile_pool(name="ps_wt", bufs=2, space="PSUM"))
        ps_acc = ctx.enter_context(tc.tile_pool(name="ps_acc", bufs=2, space="PSUM"))
        ps_sc = ctx.enter_context(tc.tile_pool(name="ps_sc", bufs=1, space="PSUM"))
        ps_se = ctx.enter_context(tc.tile_pool(name="ps_se", bufs=2, space="PSUM"))

        # ---- constants
        iota_f = cpool.tile([128, 128], dt.float32)
        nc.sync.dma_start(iota_f[:], iotaf_d)
        ws_t = cpool.tile([F, D], dt.float32)
        nc.sync.dma_start(ws_t[:], Ws_d)
        wsT_t = cpool.tile([D, F], dt.float32)
        nc.sync.dma_start(wsT_t[:], WsT_d)
        apair_t = cpool.tile([D, 2], dt.float32)
        nc.sync.dma_start(apair_t[:], apair_d)
        mask_t = cpool.tile([128, NB, Q], dt.float32)
        nc.sync.dma_start(mask_t[:], mask_d)
        srel2_t = cpool.tile([128, S2 // 128], dt.float32)
        nc.sync.dma_start(srel2_t[:], srel2_d)
        sel16_t = cpool.tile([128, CELL], dt.float32)
        nc.sync.dma_start(sel16_t[:], sel16_d)
        E16_t = cpool.tile([128, 128], dt.float32)
        nc.sync.dma_start(E16_t[:], E16_d)
        PMs_t = cpool.tile([128, NMTOT, 128], dt.bfloat16)
        nc.sync.dma_start(PMs_t[:], PMs_d)
        xtown_t = cpool.tile([F, NB, 128], dt.bfloat16)
        nc.scalar.dma_start(xtown_t[:], XTown_d.rearrange("f (b p) -> f b p", p=128))

        # Wse = [Ws@a2 | Ws] bf16: one 65-col rhs so each A1 matmul yields
        # [t_dst, Wh_dst] per slot.  Wsa1 bf16 for the s matmuls.
        wsa_ps = ps_sc.tile([128, 2], dt.float32, space="PSUM", tag="sc")
        nc.tensor.matmul(wsa_ps[:], lhsT=wsT_t[:], rhs=apair_t[:],
                         start=True, stop=True)
        Wse = cpool.tile([F, 1 + D], dt.bfloat16)
        nc.vector.tensor_copy(Wse[:, 0:1], wsa_ps[:, 0:1])
        nc.vector.tensor_copy(Wse[:, 1:1 + D], ws_t[:])
        wsa1_t = cpool.tile([F, 1], dt.bfloat16)
        nc.vector.tensor_copy(wsa1_t[:], wsa_ps[:, 1:2])

        # ---- s for own rows: s[r] = X[r] @ Ws @ a1, per block -> [128, NB]
        s_ps = ps_se.tile([128, NB], dt.float32, space="PSUM", tag="se")
        for b in range(NB):
            nc.tensor.matmul(s_ps[:, b:b + 1], lhsT=xtown_t[:, b, :],
                             rhs=wsa1_t[:], start=True, stop=True)
        sloc = cpool.tile([128, NB], dt.float32)
        nc.vector.tensor_copy(sloc[:], s_ps[:])
        # hi/lo bf16 split so the tier-2 PM matmuls stay near-f32 exact
        sloc_hl = cpool.tile([128, NB, 2], dt.bfloat16)
        nc.vector.tensor_copy(sloc_hl[:, :, 0], s_ps[:])
        nc.vector.tensor_sub(sloc_hl[:, :, 1], s_ps[:], sloc_hl[:, :, 0])

        # ---- main loop: per 128-row block
        sl2 = [0]
        for b in range(NB):
            sl2.append(sl2[-1] + Kb2[b])
        XgT1_v = XgT_d[:, 0:S1].rearrange("f (b q p) -> f b q p", q=Q, p=128)
        outstage = cpool.tile([128, NB, D], dt.float32)
        gsp = 0          # global PM span index
        for b in range(NB):
            K2 = Kb2[b]
            nm2 = ncellp2[b] // 128
            # ================= tier 1 =================
            xg1 = xgpool.tile([128, Q, 128], dt.bfloat16)
            nc.scalar.dma_start(xg1[:], XgT1_v[:, b])
            t_f = wpool.tile([128, Q], dt.float32, tag="tf")
            whp = whpool.tile([128, 1 + D, Q], dt.bfloat16)   # [Wh|1][d, q]
            nc.vector.memset(whp[:, D, :], 1.0)
            for c0 in range(0, Q, SUB):
                c1 = min(c0 + SUB, Q)
                wt_ps = ps_wt.tile([128, SUB, 1 + D], dt.float32,
                                   space="PSUM", tag="wt")
                for j in range(c1 - c0):
                    nc.tensor.matmul(wt_ps[:, j, :], lhsT=xg1[:, c0 + j, :],
                                     rhs=Wse[:], start=True, stop=True)
                nc.vector.tensor_copy(t_f[:, c0:c1], wt_ps[:, 0:c1 - c0, 0])
                nc.scalar.activation(
                    whp[:, 0:D, c0:c1],
                    wt_ps[:, 0:c1 - c0, 1:].rearrange("p q d -> p d q"),
                    mybir.ActivationFunctionType.Copy)
            # w = exp(leaky(s + t) + mask)
            e_t = wpool.tile([128, Q], dt.float32, tag="e")
            nc.vector.tensor_scalar_add(e_t[:], t_f[:], sloc[:, b:b + 1])
            lk = wpool.tile([128, Q], dt.float32, tag="lk")
            nc.vector.scalar_tensor_tensor(
                out=lk[:], in0=e_t[:], scalar=0.15, op0=mybir.AluOpType.mult,
                in1=e_t[:], op1=mybir.AluOpType.max)
            lkm = wpool.tile([128, Q], dt.float32, tag="lkm")
            nc.vector.tensor_add(lkm[:], lk[:], mask_t[:, b, :])
            w_t = wpool.tile([128, Q], dt.bfloat16, tag="wt")
            nc.scalar.activation(w_t[:], lkm[:], mybir.ActivationFunctionType.Exp)
            # G[d, q] = w[q] * [Wh, 1][d, q]; U[d] = sum_q G[d, q]
            G = Gpool.tile([128, 1 + D, Q], dt.bfloat16)
            nc.vector.tensor_mul(G[:], whp[:],
                                 w_t[:, None, :].to_broadcast([128, 1 + D, Q]))
            # pairwise bf16 fold halves the f32-out reduce's 1x work
            H = Gpool.tile([128, 1 + D, Q // 2], dt.bfloat16, tag="H")
            nc.vector.tensor_add(H[:], G[:, :, 0:Q // 2], G[:, :, Q // 2:Q])
            U_t = epool.tile([128, 1 + D], dt.float32, tag="U")
            nc.vector.reduce_sum(U_t[:], H[:], axis=mybir.AxisListType.X)
            if "dbg" in _ABL and b == 0:
                dstage = cpool.tile([128, 4 * Q + 2 * (1 + D)], dt.float32)
                nc.vector.tensor_copy(dstage[:, 0:Q], t_f[:])
                nc.vector.tensor_copy(dstage[:, Q:2 * Q], e_t[:])
                nc.vector.tensor_copy(dstage[:, 2 * Q:3 * Q], lkm[:])
                nc.vector.tensor_copy(dstage[:, 3 * Q:4 * Q], w_t[:])
                nc.vector.tensor_copy(dstage[:, 4 * Q:4 * Q + 1 + D], U_t[:])
                nc.vector.tensor_copy(
                    dstage[:, 4 * Q + 1 + D:4 * Q + 2 * (1 + D)],
                    G[:, :, 0])
                nc.sync.dma_start(dbg_d, dstage[:])
            # ================= tier 2 =================
            lo2 = S1 + sl2[b] * 128
            xg2 = x2pool.tile([128, K2, 128], dt.bfloat16)
            nc.scalar.dma_start(
                xg2[:], XgT_d[:, lo2:lo2 + K2 * 128]
                .rearrange("f (k p) -> f k p", p=128))
            t2_f = wpool.tile([128, K2], dt.float32, tag="t2")
            wh2 = whpool.tile([128, K2, 1 + D], dt.bfloat16)
            for c0 in range(0, K2, SUB):
                c1 = min(c0 + SUB, K2)
                wt_ps = ps_wt.tile([128, SUB, 1 + D], dt.float32,
                                   space="PSUM", tag="wt")
                for j in range(c1 - c0):
                    nc.tensor.matmul(wt_ps[:, j, :], lhsT=xg2[:, c0 + j, :],
                                     rhs=Wse[:], start=True, stop=True)
                nc.vector.tensor_copy(t2_f[:, c0:c1], wt_ps[:, 0:c1 - c0, 0])
                nc.scalar.activation(
                    wh2[:, c0:c1, 0:D], wt_ps[:, 0:c1 - c0, 1:],
                    mybir.ActivationFunctionType.Copy)
            # one-hot of srcrel vs row-in-block
            oht2 = ohpool.tile([128, K2, 128], dt.bfloat16)
            nc.vector.tensor_tensor(
                out=oht2[:],
                in0=iota_f[:, None, :].to_broadcast([128, K2, 128]),
                in1=srel2_t[:, sl2[b]:sl2[b] + K2, None]
                    .to_broadcast([128, K2, 128]),
                op=mybir.AluOpType.is_equal)
            # s per cell via PM one-hot matmuls, then expand cells -> slots
            sc_ps = ps_sc.tile([128, nm2], dt.float32, space="PSUM", tag="sc")
            for m in range(nm2):
                nc.tensor.matmul(sc_ps[:, m:m + 1], lhsT=PMs_t[:, gsp + m, :],
                                 rhs=sloc_hl[:, b, 0:1], start=True, stop=False)
                nc.tensor.matmul(sc_ps[:, m:m + 1], lhsT=PMs_t[:, gsp + m, :],
                                 rhs=sloc_hl[:, b, 1:2], start=False, stop=True)
            se_ps = ps_se.tile([128, nm2 * CELL], dt.float32, space="PSUM", tag="se")
            for m in range(nm2):
                rhsm = wpool.tile([128, CELL], dt.float32, tag="rhsm")
                nc.vector.tensor_mul(
                    rhsm[:], sel16_t[:],
                    sc_ps[:, m:m + 1].to_broadcast([128, CELL]))
                nc.tensor.matmul(se_ps[:, m * CELL:(m + 1) * CELL],
                                 lhsT=E16_t[:], rhs=rhsm[:],
                                 start=True, stop=True)
            e2 = wpool.tile([128, K2], dt.float32, tag="e2")
            nc.vector.tensor_add(e2[:], se_ps[:, 0:K2], t2_f[:])
            lk2 = wpool.tile([128, K2], dt.float32, tag="lk2")
            nc.vector.scalar_tensor_tensor(
                out=lk2[:], in0=e2[:], scalar=0.15, op0=mybir.AluOpType.mult,
                in1=e2[:], op1=mybir.AluOpType.max)
            w2 = wpool.tile([128, K2], dt.bfloat16, tag="w2")
            nc.scalar.activation(w2[:], lk2[:], mybir.ActivationFunctionType.Exp)
            G2 = Gpool.tile([128, K2, 1 + D], dt.bfloat16)
            nc.vector.tensor_mul(G2[:, :, 0:D], wh2[:, :, 0:D],
                                 w2[:, :, None].to_broadcast([128, K2, D]))
            nc.vector.tensor_copy(G2[:, :, D], w2[:])
            acc2 = ps_acc.tile([128, 1 + D], dt.float32, space="PSUM", tag="acc")
            for c in range(K2):
                nc.tensor.matmul(acc2[:], lhsT=oht2[:, c, :], rhs=G2[:, c, :],
                                 start=(c == 0), stop=(c == K2 - 1))
            # ================= epilogue: out = elu((U1+U2)/(Z1+Z2)) =========
            Ut = epool.tile([128, 1 + D], dt.float32, tag="Ut")
            nc.vector.tensor_add(Ut[:], U_t[:], acc2[:])
            zg = epool.tile([128, 1], dt.float32, tag="zg")
            nc.vector.tensor_scalar_max(zg[:], Ut[:, D:D + 1], 1e-30)
            zr = epool.tile([128, 1], dt.float32, tag="zr")
            nc.vector.reciprocal(zr[:], zg[:])
            x = epool.tile([128, D], dt.float32, tag="x")
            nc.vector.tensor_scalar_mul(x[:], Ut[:, 0:D], zr[:])
            mn = epool.tile([128, D], dt.float32, tag="mn")
            nc.vector.tensor_scalar_min(mn[:], x[:], 0.0)
            em = epool.tile([128, D], dt.float32, tag="em")
            nc.scalar.activation(em[:], mn[:], mybir.ActivationFunctionType.Exp)
            rl = epool.tile([128, D], dt.float32, tag="rl")
            nc.vector.tensor_scalar_max(rl[:], x[:], 0.0)
            nc.vector.scalar_tensor_tensor(
                out=outstage[:, b, :], in0=em[:], scalar=-1.0,
                op0=mybir.AluOpType.add, in1=rl[:], op1=mybir.AluOpType.add)
            gsp += nm2

        out_v = out_d.rearrange("(b p) d -> p b d", p=128)   # [128, NB, D]
        nc.sync.dma_start(out_v, outstage[:])
    nc.compile()
    return nc


_cache = {}


def _get_program(Kb2, S2, ncellp2):
    key = (tuple(Kb2), S2, tuple(ncellp2), tuple(sorted(_ABL)))
    if key not in _cache:
        _cache[key] = _build(Kb2, S2, ncellp2)
    return _cache[key]


def make_in_maps(A, X, Ws, a):
    """Host-side sharding: returns (nc, in_maps)."""
    import ml_dtypes
    X = np.asarray(X, dtype=np.float32)
    Ws = np.ascontiguousarray(np.asarray(Ws, dtype=np.float32))
    a = np.asarray(a, dtype=np.float32).reshape(2 * D)
    Xbf = X.astype(ml_dtypes.bfloat16)
    WsT = np.ascontiguousarray(Ws.T)
    apair = np.stack([a[D:], a[:D]], axis=1).astype(np.float32)  # [D, 2] = [a2|a1]
    q = np.arange(128)
    CPC = 128 // CELL
    sel16 = (q[:, None] // CPC == np.arange(CELL)[None, :]).astype(np.float32)
    E16 = (q[:, None] % CPC == q[None, :] // CELL).astype(np.float32)
    iotaf = np.tile(np.arange(128, dtype=np.float32)[None, :], (128, 1))
    cores, Kb2, S2, ncellp2 = _prep_edges(A)
    nc = _get_program(Kb2, S2, ncellp2)
    in_maps = []
    for c in range(NCORES):
        ci = cores[c]
        XgT = np.ascontiguousarray(Xbf[ci["dsti"]].T)          # [F, S1+S2]
        XTown = np.ascontiguousarray(Xbf[c * R:(c + 1) * R].T)  # [F, R]
        in_maps.append({
            "XgT": XgT, "XTown": XTown, "Ws": Ws, "WsT": WsT,
            "apair": apair, "maskneg": ci["maskneg"],
            "sel16": sel16, "E16": E16, "iotaf": iotaf,
            "srel2": ci["srel2"], "PMs": ci["PMs"],
        })
    return nc, in_maps


def kernel(A, X, Ws, a):
    nc, in_maps = make_in_maps(A, X, Ws, a)
    res = run_bass_kernel_spmd(nc, in_maps, core_ids=list(range(NCORES)),
                               trace=False)
    return np.concatenate([r["out"] for r in res.results], axis=0)


# revision 24
# speedup vs baseline: 5.1099x; 1.0889x over previous
"""GAT forward kernel for Trainium2 (8 NeuronCores, Bass/Tile).

Reference computation (dense form):
    adj = densify(A); Wh = X @ Ws; e = leaky_relu(Wh@a1 + (Wh@a2).T, 0.15)
    att = softmax(where(adj>0, e, -9e15), axis=1); out = elu(att @ Wh)

Sparse form (only ~524K of 16384^2 entries matter; |e| <= ~16 so softmax
needs no max-subtraction):
    w_e = exp(leaky(s_src + t_dst));  out_i = elu(sum_e w_e Wh_dst / sum_e w_e)

Sharding: rows (softmax queries) split 2048/core across 8 cores.

Two-tier edge layout, host pre-gathers X[dst_e] per slot (contiguous bf16
DMA, no device gather):

Tier 1 (first Q=32 edges of every row): slot (b, q, p) holds edge q of row
(b, p).  One matmul per (b, q) chunk (lhsT = XgT chunk, rhs = [Ws@a2|Ws|1])
gives [t_dst, Wh_dst, 1] for 128 rows at once, ROW-partition.  s_src is a
per-partition scalar broadcast, pad slots are masked with -1e30 before exp,
and the softmax aggregation is a plain DVE reduction over q -- no one-hot
matmuls, no PM machinery.

Tier 2 (edges Q..deg of rows with deg > Q, ~7% of slots): classic row-major
cell layout with per-chunk one-hot (is_equal vs srcrel) aggregation matmuls
and the PM cell trick for per-slot s.  Its acc joins tier 1's U in the
epilogue: out = elu((U1+U2) / (Z1+Z2)).

Host prep pads tiers to cross-core uniform chunk counts so all 8 cores run
the same program (SPMD).
"""
import os
import sys

if "/opt/trn_rl_repo" not in sys.path:
    sys.path.insert(0, "/opt/trn_rl_repo")

_ABL = set(os.environ.get("GAT_ABLATE", "").split(","))

from contextlib import ExitStack

import numpy as np

import concourse.bass as bass
import concourse.tile as tile
from concourse import bacc, mybir
from concourse.bass_utils import run_bass_kernel_spmd

N = 16384          # nodes
F = 128            # input features
D = 64             # embedding dim
NCORES = 8
R = N // NCORES    # rows per core (2048)
NB = R // 128      # row blocks per core (16)
Q = 32             # tier-1 slots per row
S1 = NB * Q * 128  # tier-1 slots per core (65536)
CELL = 8           # tier-2 slots per cell (one src row per cell)
SUB = 11           # A1 chunks per PSUM sub-batch (11*65*4B < 2 banks)
TPAD = -400.0      # pad-slot sentinel: t_pad ~ TPAD so exp(leaky) ~ 0
dt = mybir.dt


# ---------------------------------------------------------------- host prep
def _prep_edges(A):
    """Dedup edges; tier-1 = first Q edges per row in (block, q, row) slot
    order with a -inf pad mask; tier-2 = remaining edges in row-major
    CELL-padded layout, padded to cross-core uniform chunk counts Kb2."""
    src_all = np.asarray(A[0], dtype=np.int64)
    dst_all = np.asarray(A[1], dtype=np.int64)
    keys = np.unique(src_all * N + dst_all)     # dedup + sort by (src, dst)
    src = (keys // N).astype(np.int32)
    dst = (keys % N).astype(np.int32)
    E = len(dst)

    deg = np.bincount(src, minlength=N)
    assert deg.min() > 0, (
        "empty rows present; this kernel assumes every row has >=1 edge"
    )
    row_start = np.concatenate([[0], np.cumsum(deg)])

    # ---- tier 1: dsti1 [NCORES, NB, Q, 128] + pad mask (pads get the
    # sentinel X column so t_pad ~ TPAD and w vanishes without a mask op)
    rows = np.arange(N).reshape(NCORES, NB, 128)
    qs = np.arange(Q)
    pos = row_start[rows][..., None] + qs                 # [C, NB, 128, Q]
    valid = qs[None, None, None, :] < deg[rows][..., None]
    dsti1 = np.where(valid, dst[np.minimum(pos, E - 1)], 0)
    dsti1 = dsti1.transpose(0, 1, 3, 2).astype(np.int32)  # [C, NB, Q, 128]
    pad1 = (~valid).transpose(0, 1, 3, 2).reshape(NCORES, S1)

    # ---- tier 2: excess edges, row-major cells, block buckets
    exc = np.maximum(deg - Q, 0)
    excc = ((exc + CELL - 1) // CELL) * CELL
    slots_cb = excc.reshape(NCORES, NB, 128).sum(axis=2)   # [C, NB]
    Kb2 = np.maximum((slots_cb.max(axis=0) + 127) // 128, 1)   # [NB]
    S2 = int(Kb2.sum()) * 128
    offs2 = np.concatenate([[0], np.cumsum(Kb2)]) * 128
    cells_cb = slots_cb // CELL
    ncell2 = [int(k) * (128 // CELL) for k in Kb2]
    ncellp2 = [((n + 127) // 128) * 128 for n in ncell2]
    cell_offs2 = np.concatenate([[0], np.cumsum(ncellp2)])
    SC2 = int(cell_offs2[-1])

    dsti2 = np.zeros((NCORES, S2), np.int32)
    srel2 = np.full((NCORES, S2), -1.0, np.float32)
    cellsrc2 = np.zeros((NCORES, SC2), np.int16)
    for c in range(NCORES):
        for b in range(NB):
            pos2 = offs2[b]
            for p in range(128):
                r = (c * NB + b) * 128 + p
                d = int(exc[r])
                if d == 0:
                    continue
                lo = row_start[r] + Q
                dsti2[c, pos2:pos2 + d] = dst[lo:lo + d]
                srel2[c, pos2:pos2 + d] = float(p)
                ncw = int(excc[r])
                cbase = cell_offs2[b] + (pos2 - offs2[b]) // CELL
                cellsrc2[c, cbase:cbase + ncw // CELL] = r
                pos2 += ncw
            assert pos2 <= offs2[b + 1]

    import ml_dtypes
    NMTOT = sum(n // 128 for n in ncellp2)
    cores = []
    for c in range(NCORES):
        PMs = np.zeros((128, NMTOT, 128), ml_dtypes.bfloat16)
        g = 0
        for b in range(NB):
            base = (c * NB + b) * 128
            for m in range(ncellp2[b] // 128):
                cs = cellsrc2[c, cell_offs2[b] + m * 128:
                              cell_offs2[b] + (m + 1) * 128]
                rel = cs.astype(np.int64) - base
                vv = (rel >= 0) & (rel < 128)
                PMs[rel[vv], g, np.arange(128)[vv]] = 1.0
                g += 1
        assert g == NMTOT
        cores.append({
            "dsti": np.concatenate([dsti1[c].reshape(-1), dsti2[c]]),
            "pad1": pad1[c],                                     # [S1] bool
            "srel2": srel2[c].reshape(S2 // 128, 128).T.copy(),  # [128, S2/128]
            "PMs": PMs,
        })
    return cores, [int(k) for k in Kb2], S2, [int(x) for x in ncellp2]


# ---------------------------------------------------------------- device IR
def _build(Kb2, S2, ncellp2):
    SC2 = sum(ncellp2)
    NMTOT = SC2 // 128
    ST = S1 + S2
    nc = bacc.Bacc("TRN2", target_bir_lowering=False, debug=False,
                   enable_asserts=False, num_devices=NCORES,
                   num_swdge_queues=4)
    XgT_d = nc.dram_tensor("XgT", [F, ST], dt.bfloat16, kind="ExternalInput").ap()
    XTown_d = nc.dram_tensor("XTown", [F, R], dt.bfloat16, kind="ExternalInput").ap()
    Ws_d = nc.dram_tensor("Ws", [F, D], dt.float32, kind="ExternalInput").ap()
    WsT_d = nc.dram_tensor("WsT", [D, F], dt.float32, kind="ExternalInput").ap()
    apair_d = nc.dram_tensor("apair", [D, 2], dt.float32, kind="ExternalInput").ap()
    PMs_d = nc.dram_tensor("PMs", [128, NMTOT, 128], dt.bfloat16, kind="ExternalInput").ap()
    srel2_d = nc.dram_tensor("srel2", [128, S2 // 128], dt.float32, kind="ExternalInput").ap()
    sel16_d = nc.dram_tensor("sel16", [128, CELL], dt.float32, kind="ExternalInput").ap()
    E16_d = nc.dram_tensor("E16", [128, 128], dt.float32, kind="ExternalInput").ap()
    iotaf_d = nc.dram_tensor("iotaf", [128, 128], dt.float32, kind="ExternalInput").ap()
    out_d = nc.dram_tensor("out", [R, D], dt.float32, kind="ExternalOutput").ap()

    with tile.TileContext(nc) as tc, ExitStack() as ctx:
        cpool = ctx.enter_context(tc.tile_pool(name="const", bufs=1))
        xgpool = ctx.enter_context(tc.tile_pool(name="xg", bufs=3))
        x2pool = ctx.enter_context(tc.tile_pool(name="x2", bufs=3))
        whpool = ctx.enter_context(tc.tile_pool(name="wh", bufs=3))
        Gpool = ctx.enter_context(tc.tile_pool(name="G", bufs=3))
        ohpool = ctx.enter_context(tc.tile_pool(name="oh", bufs=3))
        wpool = ctx.enter_context(tc.tile_pool(name="w", bufs=3))
        epool = ctx.enter_context(tc.tile_pool(name="ep", bufs=3))
        # PSUM budget (8 banks): wt 2x1, acc2 2x1, sc 1, se 2x1
        ps_wt = ctx.enter_context(tc.tile_pool(name="ps_wt", bufs=2, space="PSUM"))
        ps_acc = ctx.enter_context(tc.tile_pool(name="ps_acc", bufs=2, space="PSUM"))
        ps_sc = ctx.enter_context(tc.tile_pool(name="ps_sc", bufs=1, space="PSUM"))
        ps_se = ctx.enter_context(tc.tile_pool(name="ps_se", bufs=1, space="PSUM"))

        # ---- constants
        iota_f = cpool.tile([128, 128], dt.float32)
        nc.sync.dma_start(iota_f[:], iotaf_d)
        ws_t = cpool.tile([F, D], dt.float32)
        nc.sync.dma_start(ws_t[:], Ws_d)
        wsT_t = cpool.tile([D, F], dt.float32)
        nc.sync.dma_start(wsT_t[:], WsT_d)
        apair_t = cpool.tile([D, 2], dt.float32)
        nc.sync.dma_start(apair_t[:], apair_d)
        srel2_t = cpool.tile([128, S2 // 128], dt.float32)
        nc.sync.dma_start(srel2_t[:], srel2_d)
        sel16_t = cpool.tile([128, CELL], dt.float32)
        nc.sync.dma_start(sel16_t[:], sel16_d)
        E16_t = cpool.tile([128, 128], dt.float32)
        nc.sync.dma_start(E16_t[:], E16_d)
        PMs_t = cpool.tile([128, NMTOT, 128], dt.bfloat16)
        nc.sync.dma_start(PMs_t[:], PMs_d)
        xtown_t = cpool.tile([F, NB, 128], dt.bfloat16)
        nc.scalar.dma_start(xtown_t[:], XTown_d.rearrange("f (b p) -> f b p", p=128))

        # Wse = [Ws@a2 | Ws] bf16: one 65-col rhs so each A1 matmul yields
        # [t_dst, Wh_dst] per slot.  Wsa1 bf16 for the s matmuls.
        wsa_ps = ps_sc.tile([128, 2], dt.float32, space="PSUM", tag="sc")
        nc.tensor.matmul(wsa_ps[:], lhsT=wsT_t[:], rhs=apair_t[:],
                         start=True, stop=True)
        Wse = cpool.tile([F, 1 + D], dt.bfloat16)
        nc.vector.tensor_copy(Wse[:, 0:1], wsa_ps[:, 0:1])
        nc.vector.tensor_copy(Wse[:, 1:1 + D], ws_t[:])
        wsa1_t = cpool.tile([F, 1], dt.bfloat16)
        nc.vector.tensor_copy(wsa1_t[:], wsa_ps[:, 1:2])

        # ---- s for own rows: s[r] = X[r] @ Ws @ a1, per block -> [128, NB]
        s_ps = ps_se.tile([128, NB], dt.float32, space="PSUM", tag="se")
        for b in range(NB):
            nc.tensor.matmul(s_ps[:, b:b + 1], lhsT=xtown_t[:, b, :],
                             rhs=wsa1_t[:], start=True, stop=True)
        sloc = cpool.tile([128, NB], dt.float32)
        nc.vector.tensor_copy(sloc[:], s_ps[:])
        # hi/lo bf16 split so the tier-2 PM matmuls stay near-f32 exact
        sloc_hl = cpool.tile([128, NB, 2], dt.bfloat16)
        nc.vector.tensor_copy(sloc_hl[:, :, 0], s_ps[:])
        nc.vector.tensor_sub(sloc_hl[:, :, 1], s_ps[:], sloc_hl[:, :, 0])

        # ---- main loop: per 128-row block
        sl2 = [0]
        for b in range(NB):
            sl2.append(sl2[-1] + Kb2[b])
        XgT1_v = XgT_d[:, 0:S1].rearrange("f (b q p) -> f b q p", q=Q, p=128)
        outstage = cpool.tile([128, NB, D], dt.float32)
        gsp = 0          # global PM span index
        for b in range(NB):
            K2 = Kb2[b]
            nm2 = ncellp2[b] // 128
            # ================= tier 1 =================
            xg1 = xgpool.tile([128, Q, 128], dt.bfloat16)
            nc.scalar.dma_start(xg1[:], XgT1_v[:, b])
            t_f = wpool.tile([128, Q], dt.float32, tag="tf")
            whp = whpool.tile([128, 1 + D, Q], dt.bfloat16)   # [Wh|1][d, q]
            nc.vector.memset(whp[:, D, :], 1.0)
            # sub-batches through a 2-bank PSUM tile: 7 chunks of 65 per bank
            # (matmul outputs must stay inside one 512-f32 bank)
            c0 = 0
            for nb, ns in ((2, 7), (2, 7), (1, 4)):
                n = nb * ns
                wt_ps = ps_wt.tile([128, 2, 512], dt.float32,
                                   space="PSUM", tag="wt")
                for j in range(n):
                    nc.tensor.matmul(
                        wt_ps[:, j // ns, (j % ns) * 65:(j % ns) * 65 + 65],
                        lhsT=xg1[:, c0 + j, :],
                        rhs=Wse[:], start=True, stop=True)
                wt_v = wt_ps[:, 0:nb, 0:ns * 65].rearrange(
                    "p b (s d) -> p b s d", d=65)
                nc.vector.tensor_copy(
                    t_f[:, c0:c0 + n].rearrange("p (b s) -> p b s", s=ns),
                    wt_v[:, :, :, 0])
                nc.scalar.activation(
                    whp[:, 0:D, c0:c0 + n].rearrange("p d (b s) -> p d b s", s=ns),
                    wt_v[:, :, :, 1:].rearrange("p b s d -> p d b s"),
                    mybir.ActivationFunctionType.Copy)
                c0 += n
            # w = exp(leaky(s + t) + mask)
            e_t = wpool.tile([128, Q], dt.float32, tag="e")
            nc.vector.tensor_scalar_add(e_t[:], t_f[:], sloc[:, b:b + 1])
            lk = wpool.tile([128, Q], dt.float32, tag="lk")
            nc.vector.scalar_tensor_tensor(
                out=lk[:], in0=e_t[:], scalar=0.15, op0=mybir.AluOpType.mult,
                in1=e_t[:], op1=mybir.AluOpType.max)
            w_t = wpool.tile([128, Q], dt.bfloat16, tag="wt")
            nc.scalar.activation(w_t[:], lk[:], mybir.ActivationFunctionType.Exp)
            # G[d, q] = w[q] * [Wh, 1][d, q]; U[d] = sum_q G[d, q]
            G = Gpool.tile([128, 1 + D, Q], dt.bfloat16)
            nc.vector.tensor_mul(G[:], whp[:],
                                 w_t[:, None, :].to_broadcast([128, 1 + D, Q]))
            # two pairwise bf16 folds quarter the f32-out reduce's 1x work
            H = Gpool.tile([128, 1 + D, Q // 2], dt.bfloat16, tag="H")
            nc.vector.tensor_add(H[:], G[:, :, 0:Q // 2], G[:, :, Q // 2:Q])
            H2 = Gpool.tile([128, 1 + D, Q // 4], dt.bfloat16, tag="H2")
            nc.vector.tensor_add(H2[:], H[:, :, 0:Q // 4], H[:, :, Q // 4:Q // 2])
            U_t = epool.tile([128, 1 + D], dt.float32, tag="U")
            nc.vector.reduce_sum(U_t[:], H2[:], axis=mybir.AxisListType.X)
            # ================= tier 2 =================
            lo2 = S1 + sl2[b] * 128
            xg2 = x2pool.tile([128, K2, 128], dt.bfloat16)
            nc.scalar.dma_start(
                xg2[:], XgT_d[:, lo2:lo2 + K2 * 128]
                .rearrange("f (k p) -> f k p", p=128))
            t2_f = wpool.tile([128, K2], dt.float32, tag="t2")
            wh2 = whpool.tile([128, K2, 1 + D], dt.bfloat16)
            for c0 in range(0, K2, 7):
                c1 = min(c0 + 7, K2)
                wt_ps = ps_wt.tile([128, 2, 512], dt.float32,
                                   space="PSUM", tag="wt")
                for j in range(c1 - c0):
                    nc.tensor.matmul(wt_ps[:, 0, j * 65:j * 65 + 65],
                                     lhsT=xg2[:, c0 + j, :],
                                     rhs=Wse[:], start=True, stop=True)
                wt2_v = wt_ps[:, 0, 0:(c1 - c0) * 65].rearrange(
                    "p (s d) -> p s d", d=65)
                nc.vector.tensor_copy(t2_f[:, c0:c1], wt2_v[:, :, 0])
                nc.scalar.activation(
                    wh2[:, c0:c1, 0:D], wt2_v[:, :, 1:],
                    mybir.ActivationFunctionType.Copy)
            # one-hot of srcrel vs row-in-block
            oht2 = ohpool.tile([128, K2, 128], dt.bfloat16)
            nc.vector.tensor_tensor(
                out=oht2[:],
                in0=iota_f[:, None, :].to_broadcast([128, K2, 128]),
                in1=srel2_t[:, sl2[b]:sl2[b] + K2, None]
                    .to_broadcast([128, K2, 128]),
                op=mybir.AluOpType.is_equal)
            # s per cell via PM one-hot matmuls, then expand cells -> slots
            sc_ps = ps_sc.tile([128, nm2], dt.float32, space="PSUM", tag="sc")
            for m in range(nm2):
                nc.tensor.matmul(sc_ps[:, m:m + 1], lhsT=PMs_t[:, gsp + m, :],
                                 rhs=sloc_hl[:, b, 0:1], start=True, stop=False)
                nc.tensor.matmul(sc_ps[:, m:m + 1], lhsT=PMs_t[:, gsp + m, :],
                                 rhs=sloc_hl[:, b, 1:2], start=False, stop=True)
            se_ps = ps_se.tile([128, nm2 * CELL], dt.float32, space="PSUM", tag="se")
            for m in range(nm2):
                rhsm = wpool.tile([128, CELL], dt.float32, tag="rhsm")
                nc.vector.tensor_mul(
                    rhsm[:], sel16_t[:],
                    sc_ps[:, m:m + 1].to_broadcast([128, CELL]))
                nc.tensor.matmul(se_ps[:, m * CELL:(m + 1) * CELL],
                                 lhsT=E16_t[:], rhs=rhsm[:],
                                 start=True, stop=True)
            e2 = wpool.tile([128, K2], dt.float32, tag="e2")
            nc.vector.tensor_add(e2[:], se_ps[:, 0:K2], t2_f[:])
            lk2 = wpool.tile([128, K2], dt.float32, tag="lk2")
            nc.vector.scalar_tensor_tensor(
                out=lk2[:], in0=e2[:], scalar=0.15, op0=mybir.AluOpType.mult,
                in1=e2[:], op1=mybir.AluOpType.max)
            w2 = wpool.tile([128, K2], dt.bfloat16, tag="w2")
            nc.scalar.activation(w2[:], lk2[:], mybir.ActivationFunctionType.Exp)
            G2 = Gpool.tile([128, K2, 1 + D], dt.bfloat16)
            nc.vector.tensor_mul(G2[:, :, 0:D], wh2[:, :, 0:D],
                                 w2[:, :, None].to_broadcast([128, K2, D]))
            nc.vector.tensor_copy(G2[:, :, D], w2[:])
            acc2 = ps_acc.tile([128, 1 + D], dt.float32, space="PSUM", tag="acc")
            for c in range(K2):
                nc.tensor.matmul(acc2[:], lhsT=oht2[:, c, :], rhs=G2[:, c, :],
                                 start=(c == 0), stop=(c == K2 - 1))
            # ================= epilogue: out = elu((U1+U2)/(Z1+Z2)) =========
            Ut = epool.tile([128, 1 + D], dt.float32, tag="Ut")
            nc.vector.tensor_add(Ut[:], U_t[:], acc2[:])
            zg = epool.tile([128, 1], dt.float32, tag="zg")
            nc.vector.tensor_scalar_max(zg[:], Ut[:, D:D + 1], 1e-30)
            zr = epool.tile([128, 1], dt.float32, tag="zr")
            nc.vector.reciprocal(zr[:], zg[:])
            x = epool.tile([128, D], dt.float32, tag="x")
            nc.vector.tensor_scalar_mul(x[:], Ut[:, 0:D], zr[:])
            mn = epool.tile([128, D], dt.float32, tag="mn")
            nc.vector.tensor_scalar_min(mn[:], x[:], 0.0)
            em = epool.tile([128, D], dt.float32, tag="em")
            nc.scalar.activation(em[:], mn[:], mybir.ActivationFunctionType.Exp)
            rl = epool.tile([128, D], dt.float32, tag="rl")
            nc.vector.tensor_scalar_max(rl[:], x[:], 0.0)
            nc.vector.scalar_tensor_tensor(
                out=outstage[:, b, :], in0=em[:], scalar=-1.0,
                op0=mybir.AluOpType.add, in1=rl[:], op1=mybir.AluOpType.add)
            gsp += nm2

        out_v = out_d.rearrange("(b p) d -> p b d", p=128)   # [128, NB, D]
        nc.sync.dma_start(out_v, outstage[:])
    nc.compile()
    return nc


_cache = {}


def _get_program(Kb2, S2, ncellp2):
    key = (tuple(Kb2), S2, tuple(ncellp2), tuple(sorted(_ABL)))
    if key not in _cache:
        _cache[key] = _build(Kb2, S2, ncellp2)
    return _cache[key]


def make_in_maps(A, X, Ws, a):
    """Host-side sharding: returns (nc, in_maps)."""
    import ml_dtypes
    X = np.asarray(X, dtype=np.float32)
    Ws = np.ascontiguousarray(np.asarray(Ws, dtype=np.float32))
    a = np.asarray(a, dtype=np.float32).reshape(2 * D)
    Xbf = X.astype(ml_dtypes.bfloat16)
    WsT = np.ascontiguousarray(Ws.T)
    apair = np.stack([a[D:], a[:D]], axis=1).astype(np.float32)  # [D, 2] = [a2|a1]
    q = np.arange(128)
    CPC = 128 // CELL
    sel16 = (q[:, None] // CPC == np.arange(CELL)[None, :]).astype(np.float32)
    E16 = (q[:, None] % CPC == q[None, :] // CELL).astype(np.float32)
    iotaf = np.tile(np.arange(128, dtype=np.float32)[None, :], (128, 1))
    cores, Kb2, S2, ncellp2 = _prep_edges(A)
    nc = _get_program(Kb2, S2, ncellp2)
    # pad sentinel column: v . (Ws@a2) = TPAD so pad slots get t ~ TPAD and
    # w = exp(leaky(s+t)) ~ e^-58 ~ 0 with no mask op on device
    wsa2 = Ws @ a[D:]
    nrm = float((wsa2 ** 2).sum())
    assert nrm > 1e-8, "degenerate Ws@a2; sentinel padding invalid"
    vpad = (TPAD / nrm) * wsa2
    vpad_bf = vpad.astype(ml_dtypes.bfloat16)
    in_maps = []
    for c in range(NCORES):
        ci = cores[c]
        Xg = Xbf[ci["dsti"]]                                    # [S1+S2, F]
        Xg[:S1][ci["pad1"]] = vpad_bf
        XgT = np.ascontiguousarray(Xg.T)                        # [F, S1+S2]
        XTown = np.ascontiguousarray(Xbf[c * R:(c + 1) * R].T)  # [F, R]
        in_maps.append({
            "XgT": XgT, "XTown": XTown, "Ws": Ws, "WsT": WsT,
            "apair": apair,
            "sel16": sel16, "E16": E16, "iotaf": iotaf,
            "srel2": ci["srel2"], "PMs": ci["PMs"],
        })
    return nc, in_maps


def kernel(A, X, Ws, a):
    nc, in_maps = make_in_maps(A, X, Ws, a)
    res = run_bass_kernel_spmd(nc, in_maps, core_ids=list(range(NCORES)),
                               trace=False)
    return np.concatenate([r["out"] for r in res.results], axis=0)


# revision 25
# speedup vs baseline: 5.3186x; 1.0408x over previous
"""GAT forward kernel for Trainium2 (8 NeuronCores, Bass/Tile).

Reference computation (dense form):
    adj = densify(A); Wh = X @ Ws; e = leaky_relu(Wh@a1 + (Wh@a2).T, 0.15)
    att = softmax(where(adj>0, e, -9e15), axis=1); out = elu(att @ Wh)

Sparse form (only ~524K of 16384^2 entries matter; |e| <= ~16 so softmax
needs no max-subtraction):
    w_e = exp(leaky(s_src + t_dst));  out_i = elu(sum_e w_e Wh_dst / sum_e w_e)

Sharding: rows (softmax queries) split 2048/core across 8 cores.

Two-tier edge layout, host pre-gathers X[dst_e] per slot (contiguous bf16
DMA, no device gather):

Tier 1 (first Q=32 edges of every row): slot (b, q, p) holds edge q of row
(b, p).  One matmul per (b, q) chunk (lhsT = XgT chunk, rhs = [Ws@a2|Ws|1])
gives [t_dst, Wh_dst, 1] for 128 rows at once, ROW-partition.  s_src is a
per-partition scalar broadcast, pad slots are masked with -1e30 before exp,
and the softmax aggregation is a plain DVE reduction over q -- no one-hot
matmuls, no PM machinery.

Tier 2 (edges Q..deg of rows with deg > Q, ~7% of slots): classic row-major
cell layout with per-chunk one-hot (is_equal vs srcrel) aggregation matmuls
and the PM cell trick for per-slot s.  Its acc joins tier 1's U in the
epilogue: out = elu((U1+U2) / (Z1+Z2)).

Host prep pads tiers to cross-core uniform chunk counts so all 8 cores run
the same program (SPMD).
"""
import os
import sys

if "/opt/trn_rl_repo" not in sys.path:
    sys.path.insert(0, "/opt/trn_rl_repo")

_ABL = set(os.environ.get("GAT_ABLATE", "").split(","))

from contextlib import ExitStack

import numpy as np

import concourse.bass as bass
import concourse.tile as tile
from concourse import bacc, mybir
from concourse.bass_utils import run_bass_kernel_spmd

N = 16384          # nodes
F = 128            # input features
D = 64             # embedding dim
NCORES = 8
R = N // NCORES    # rows per core (2048)
NB = R // 128      # row blocks per core (16)
Q = 32             # tier-1 slots per row
S1 = NB * Q * 128  # tier-1 slots per core (65536)
CELL = 8           # tier-2 slots per cell (one src row per cell)
SUB = 11           # A1 chunks per PSUM sub-batch (11*65*4B < 2 banks)
TPAD = -400.0      # pad-slot sentinel: t_pad ~ TPAD so exp(leaky) ~ 0
dt = mybir.dt


# ---------------------------------------------------------------- host prep
def _prep_edges(A):
    """Dedup edges; tier-1 = first Q edges per row in (block, q, row) slot
    order with a -inf pad mask; tier-2 = remaining edges in row-major
    CELL-padded layout, padded to cross-core uniform chunk counts Kb2."""
    src_all = np.asarray(A[0], dtype=np.int64)
    dst_all = np.asarray(A[1], dtype=np.int64)
    keys = np.unique(src_all * N + dst_all)     # dedup + sort by (src, dst)
    src = (keys // N).astype(np.int32)
    dst = (keys % N).astype(np.int32)
    E = len(dst)

    deg = np.bincount(src, minlength=N)
    assert deg.min() > 0, (
        "empty rows present; this kernel assumes every row has >=1 edge"
    )
    row_start = np.concatenate([[0], np.cumsum(deg)])

    # ---- tier 1: dsti1 [NCORES, NB, Q, 128] + pad mask (pads get the
    # sentinel X column so t_pad ~ TPAD and w vanishes without a mask op)
    rows = np.arange(N).reshape(NCORES, NB, 128)
    qs = np.arange(Q)
    pos = row_start[rows][..., None] + qs                 # [C, NB, 128, Q]
    valid = qs[None, None, None, :] < deg[rows][..., None]
    dsti1 = np.where(valid, dst[np.minimum(pos, E - 1)], 0)
    dsti1 = dsti1.transpose(0, 1, 3, 2).astype(np.int32)  # [C, NB, Q, 128]
    pad1 = (~valid).transpose(0, 1, 3, 2).reshape(NCORES, S1)

    # ---- tier 2: excess edges, row-major cells, block buckets
    exc = np.maximum(deg - Q, 0)
    excc = ((exc + CELL - 1) // CELL) * CELL
    slots_cb = excc.reshape(NCORES, NB, 128).sum(axis=2)   # [C, NB]
    Kb2 = np.maximum((slots_cb.max(axis=0) + 127) // 128, 1)   # [NB]
    S2 = int(Kb2.sum()) * 128
    offs2 = np.concatenate([[0], np.cumsum(Kb2)]) * 128
    cells_cb = slots_cb // CELL
    ncell2 = [int(k) * (128 // CELL) for k in Kb2]
    ncellp2 = [((n + 127) // 128) * 128 for n in ncell2]
    cell_offs2 = np.concatenate([[0], np.cumsum(ncellp2)])
    SC2 = int(cell_offs2[-1])

    dsti2 = np.zeros((NCORES, S2), np.int32)
    srel2 = np.full((NCORES, S2), -1.0, np.float32)
    cellsrc2 = np.zeros((NCORES, SC2), np.int16)
    for c in range(NCORES):
        for b in range(NB):
            pos2 = offs2[b]
            for p in range(128):
                r = (c * NB + b) * 128 + p
                d = int(exc[r])
                if d == 0:
                    continue
                lo = row_start[r] + Q
                dsti2[c, pos2:pos2 + d] = dst[lo:lo + d]
                srel2[c, pos2:pos2 + d] = float(p)
                ncw = int(excc[r])
                cbase = cell_offs2[b] + (pos2 - offs2[b]) // CELL
                cellsrc2[c, cbase:cbase + ncw // CELL] = r
                pos2 += ncw
            assert pos2 <= offs2[b + 1]

    import ml_dtypes
    NMTOT = sum(n // 128 for n in ncellp2)
    cores = []
    for c in range(NCORES):
        PMs = np.zeros((128, NMTOT, 128), ml_dtypes.bfloat16)
        g = 0
        for b in range(NB):
            base = (c * NB + b) * 128
            for m in range(ncellp2[b] // 128):
                cs = cellsrc2[c, cell_offs2[b] + m * 128:
                              cell_offs2[b] + (m + 1) * 128]
                rel = cs.astype(np.int64) - base
                vv = (rel >= 0) & (rel < 128)
                PMs[rel[vv], g, np.arange(128)[vv]] = 1.0
                g += 1
        assert g == NMTOT
        cores.append({
            "dsti": np.concatenate([dsti1[c].reshape(-1), dsti2[c]]),
            "pad1": pad1[c],                                     # [S1] bool
            "srel2": srel2[c].reshape(S2 // 128, 128).T.copy(),  # [128, S2/128]
            "PMs": PMs,
        })
    return cores, [int(k) for k in Kb2], S2, [int(x) for x in ncellp2]


# ---------------------------------------------------------------- device IR
def _build(Kb2, S2, ncellp2):
    SC2 = sum(ncellp2)
    NMTOT = SC2 // 128
    ST = S1 + S2
    nc = bacc.Bacc("TRN2", target_bir_lowering=False, debug=False,
                   enable_asserts=False, num_devices=NCORES,
                   num_swdge_queues=4)
    XgT_d = nc.dram_tensor("XgT", [F, ST], dt.bfloat16, kind="ExternalInput").ap()
    XTown_d = nc.dram_tensor("XTown", [F, R], dt.bfloat16, kind="ExternalInput").ap()
    Ws_d = nc.dram_tensor("Ws", [F, D], dt.float32, kind="ExternalInput").ap()
    WsT_d = nc.dram_tensor("WsT", [D, F], dt.float32, kind="ExternalInput").ap()
    apair_d = nc.dram_tensor("apair", [D, 2], dt.float32, kind="ExternalInput").ap()
    PMs_d = nc.dram_tensor("PMs", [128, NMTOT, 128], dt.bfloat16, kind="ExternalInput").ap()
    srel2_d = nc.dram_tensor("srel2", [128, S2 // 128], dt.float32, kind="ExternalInput").ap()
    sel16_d = nc.dram_tensor("sel16", [128, CELL], dt.float32, kind="ExternalInput").ap()
    E16_d = nc.dram_tensor("E16", [128, 128], dt.float32, kind="ExternalInput").ap()
    iotaf_d = nc.dram_tensor("iotaf", [128, 128], dt.float32, kind="ExternalInput").ap()
    out_d = nc.dram_tensor("out", [R, D], dt.float32, kind="ExternalOutput").ap()

    with tile.TileContext(nc) as tc, ExitStack() as ctx:
        cpool = ctx.enter_context(tc.tile_pool(name="const", bufs=1))
        xgpool = ctx.enter_context(tc.tile_pool(name="xg", bufs=3))
        x2pool = ctx.enter_context(tc.tile_pool(name="x2", bufs=3))
        whpool = ctx.enter_context(tc.tile_pool(name="wh", bufs=3))
        Gpool = ctx.enter_context(tc.tile_pool(name="G", bufs=3))
        ohpool = ctx.enter_context(tc.tile_pool(name="oh", bufs=3))
        wpool = ctx.enter_context(tc.tile_pool(name="w", bufs=3))
        epool = ctx.enter_context(tc.tile_pool(name="ep", bufs=3))
        # PSUM budget (8 banks): wt 2x1, acc2 2x1, sc 1, se 2x1
        ps_wt = ctx.enter_context(tc.tile_pool(name="ps_wt", bufs=2, space="PSUM"))
        ps_acc = ctx.enter_context(tc.tile_pool(name="ps_acc", bufs=2, space="PSUM"))
        ps_sc = ctx.enter_context(tc.tile_pool(name="ps_sc", bufs=1, space="PSUM"))
        ps_se = ctx.enter_context(tc.tile_pool(name="ps_se", bufs=1, space="PSUM"))

        # ---- constants
        iota_f = cpool.tile([128, 128], dt.float32)
        nc.sync.dma_start(iota_f[:], iotaf_d)
        ws_t = cpool.tile([F, D], dt.float32)
        nc.sync.dma_start(ws_t[:], Ws_d)
        wsT_t = cpool.tile([D, F], dt.float32)
        nc.sync.dma_start(wsT_t[:], WsT_d)
        apair_t = cpool.tile([D, 2], dt.float32)
        nc.sync.dma_start(apair_t[:], apair_d)
        srel2_t = cpool.tile([128, S2 // 128], dt.float32)
        nc.sync.dma_start(srel2_t[:], srel2_d)
        sel16_t = cpool.tile([128, CELL], dt.float32)
        nc.sync.dma_start(sel16_t[:], sel16_d)
        E16_t = cpool.tile([128, 128], dt.float32)
        nc.sync.dma_start(E16_t[:], E16_d)
        PMs_t = cpool.tile([128, NMTOT, 128], dt.bfloat16)
        nc.sync.dma_start(PMs_t[:], PMs_d)
        xtown_t = cpool.tile([F, NB, 128], dt.bfloat16)
        nc.scalar.dma_start(xtown_t[:], XTown_d.rearrange("f (b p) -> f b p", p=128))

        # Wse = [Ws@a2 | Ws] bf16: one 65-col rhs so each A1 matmul yields
        # [t_dst, Wh_dst] per slot.  Wsa1 bf16 for the s matmuls.
        wsa_ps = ps_sc.tile([128, 2], dt.float32, space="PSUM", tag="sc")
        nc.tensor.matmul(wsa_ps[:], lhsT=wsT_t[:], rhs=apair_t[:],
                         start=True, stop=True)
        Wse = cpool.tile([F, 1 + D], dt.bfloat16)
        nc.vector.tensor_copy(Wse[:, 0:1], wsa_ps[:, 0:1])
        nc.vector.tensor_copy(Wse[:, 1:1 + D], ws_t[:])
        wsa1_t = cpool.tile([F, 1], dt.bfloat16)
        nc.vector.tensor_copy(wsa1_t[:], wsa_ps[:, 1:2])

        # ---- s for own rows: s[r] = X[r] @ Ws @ a1, per block -> [128, NB]
        s_ps = ps_se.tile([128, NB], dt.float32, space="PSUM", tag="se")
        for b in range(NB):
            nc.tensor.matmul(s_ps[:, b:b + 1], lhsT=xtown_t[:, b, :],
                             rhs=wsa1_t[:], start=True, stop=True)
        sloc = cpool.tile([128, NB], dt.float32)
        nc.vector.tensor_copy(sloc[:], s_ps[:])
        # hi/lo bf16 split so the tier-2 PM matmuls stay near-f32 exact
        sloc_hl = cpool.tile([128, NB, 2], dt.bfloat16)
        nc.vector.tensor_copy(sloc_hl[:, :, 0], s_ps[:])
        nc.vector.tensor_sub(sloc_hl[:, :, 1], s_ps[:], sloc_hl[:, :, 0])

        # ---- main loop: per 128-row block
        sl2 = [0]
        for b in range(NB):
            sl2.append(sl2[-1] + Kb2[b])
        XgT1_v = XgT_d[:, 0:S1].rearrange("f (b q p) -> f b q p", q=Q, p=128)
        outstage = cpool.tile([128, NB, D], dt.float32)
        gsp = 0          # global PM span index
        for b in range(NB):
            K2 = Kb2[b]
            nm2 = ncellp2[b] // 128
            # ================= tier 1 =================
            xg1 = xgpool.tile([128, Q, 128], dt.bfloat16)
            nc.scalar.dma_start(xg1[:], XgT1_v[:, b])
            t_f = wpool.tile([128, Q], dt.float32, tag="tf")
            whp = whpool.tile([128, 1 + D, Q], dt.bfloat16)   # [Wh|1][d, q]
            nc.vector.memset(whp[:, D, :], 1.0)
            # sub-batches through a 2-bank PSUM tile: 7 chunks of 65 per bank
            # (matmul outputs must stay inside one 512-f32 bank)
            c0 = 0
            for nb, ns in ((2, 7), (2, 7), (1, 4)):
                n = nb * ns
                wt_ps = ps_wt.tile([128, 2, 512], dt.float32,
                                   space="PSUM", tag="wt")
                for j in range(n):
                    nc.tensor.matmul(
                        wt_ps[:, j // ns, (j % ns) * 65:(j % ns) * 65 + 65],
                        lhsT=xg1[:, c0 + j, :],
                        rhs=Wse[:], start=True, stop=True)
                wt_v = wt_ps[:, 0:nb, 0:ns * 65].rearrange(
                    "p b (s d) -> p b s d", d=65)
                nc.vector.tensor_copy(
                    t_f[:, c0:c0 + n].rearrange("p (b s) -> p b s", s=ns),
                    wt_v[:, :, :, 0])
                nc.scalar.activation(
                    whp[:, 0:D, c0:c0 + n].rearrange("p d (b s) -> p d b s", s=ns),
                    wt_v[:, :, :, 1:].rearrange("p b s d -> p d b s"),
                    mybir.ActivationFunctionType.Copy)
                c0 += n
            # w = exp(leaky(s + t) + mask)
            e_t = wpool.tile([128, Q], dt.float32, tag="e")
            nc.vector.tensor_scalar_add(e_t[:], t_f[:], sloc[:, b:b + 1])
            lk = wpool.tile([128, Q], dt.float32, tag="lk")
            nc.vector.scalar_tensor_tensor(
                out=lk[:], in0=e_t[:], scalar=0.15, op0=mybir.AluOpType.mult,
                in1=e_t[:], op1=mybir.AluOpType.max)
            w_t = wpool.tile([128, Q], dt.bfloat16, tag="wt")
            nc.scalar.activation(w_t[:], lk[:], mybir.ActivationFunctionType.Exp)
            # G[d, q] = w[q] * [Wh, 1][d, q]; U[d] = sum_q G[d, q]
            G = Gpool.tile([128, 1 + D, Q], dt.bfloat16)
            nc.vector.tensor_mul(G[:], whp[:],
                                 w_t[:, None, :].to_broadcast([128, 1 + D, Q]))
            # two pairwise bf16 folds quarter the f32-out reduce's 1x work
            H = Gpool.tile([128, 1 + D, Q // 2], dt.bfloat16, tag="H")
            nc.vector.tensor_add(H[:], G[:, :, 0:Q // 2], G[:, :, Q // 2:Q])
            H2 = Gpool.tile([128, 1 + D, Q // 4], dt.bfloat16, tag="H2")
            nc.vector.tensor_add(H2[:], H[:, :, 0:Q // 4], H[:, :, Q // 4:Q // 2])
            U_t = epool.tile([128, 1 + D], dt.float32, tag="U")
            nc.vector.reduce_sum(U_t[:], H2[:], axis=mybir.AxisListType.X)
            # ================= tier 2 =================
            lo2 = S1 + sl2[b] * 128
            xg2 = x2pool.tile([128, K2, 128], dt.bfloat16)
            nc.scalar.dma_start(
                xg2[:], XgT_d[:, lo2:lo2 + K2 * 128]
                .rearrange("f (k p) -> f k p", p=128))
            t2_f = wpool.tile([128, K2], dt.float32, tag="t2")
            wh2 = whpool.tile([128, K2, 1 + D], dt.bfloat16)
            for c0 in range(0, K2, 7):
                c1 = min(c0 + 7, K2)
                wt_ps = ps_wt.tile([128, 2, 512], dt.float32,
                                   space="PSUM", tag="wt")
                for j in range(c1 - c0):
                    nc.tensor.matmul(wt_ps[:, 0, j * 65:j * 65 + 65],
                                     lhsT=xg2[:, c0 + j, :],
                                     rhs=Wse[:], start=True, stop=True)
                wt2_v = wt_ps[:, 0, 0:(c1 - c0) * 65].rearrange(
                    "p (s d) -> p s d", d=65)
                nc.vector.tensor_copy(t2_f[:, c0:c1], wt2_v[:, :, 0])
                nc.scalar.activation(
                    wh2[:, c0:c1, 0:D], wt2_v[:, :, 1:],
                    mybir.ActivationFunctionType.Copy)
            # one-hot of srcrel vs row-in-block
            oht2 = ohpool.tile([128, K2, 128], dt.bfloat16)
            nc.vector.tensor_tensor(
                out=oht2[:],
                in0=iota_f[:, None, :].to_broadcast([128, K2, 128]),
                in1=srel2_t[:, sl2[b]:sl2[b] + K2, None]
                    .to_broadcast([128, K2, 128]),
                op=mybir.AluOpType.is_equal)
            # s per cell via PM one-hot matmuls, then expand cells -> slots
            sc_ps = ps_sc.tile([128, nm2], dt.float32, space="PSUM", tag="sc")
            for m in range(nm2):
                nc.tensor.matmul(sc_ps[:, m:m + 1], lhsT=PMs_t[:, gsp + m, :],
                                 rhs=sloc_hl[:, b, 0:1], start=True, stop=False)
                nc.tensor.matmul(sc_ps[:, m:m + 1], lhsT=PMs_t[:, gsp + m, :],
                                 rhs=sloc_hl[:, b, 1:2], start=False, stop=True)
            se_ps = ps_se.tile([128, nm2 * CELL], dt.float32, space="PSUM", tag="se")
            for m in range(nm2):
                rhsm = wpool.tile([128, CELL], dt.float32, tag="rhsm")
                nc.vector.tensor_mul(
                    rhsm[:], sel16_t[:],
                    sc_ps[:, m:m + 1].to_broadcast([128, CELL]))
                nc.tensor.matmul(se_ps[:, m * CELL:(m + 1) * CELL],
                                 lhsT=E16_t[:], rhs=rhsm[:],
                                 start=True, stop=True)
            e2 = wpool.tile([128, K2], dt.float32, tag="e2")
            nc.vector.tensor_add(e2[:], se_ps[:, 0:K2], t2_f[:])
            lk2 = wpool.tile([128, K2], dt.float32, tag="lk2")
            nc.vector.scalar_tensor_tensor(
                out=lk2[:], in0=e2[:], scalar=0.15, op0=mybir.AluOpType.mult,
                in1=e2[:], op1=mybir.AluOpType.max)
            w2 = wpool.tile([128, K2], dt.bfloat16, tag="w2")
            nc.scalar.activation(w2[:], lk2[:], mybir.ActivationFunctionType.Exp)
            G2 = Gpool.tile([128, K2, 1 + D], dt.bfloat16)
            nc.vector.tensor_mul(G2[:, :, 0:D], wh2[:, :, 0:D],
                                 w2[:, :, None].to_broadcast([128, K2, D]))
            nc.vector.tensor_copy(G2[:, :, D], w2[:])
            acc2 = ps_acc.tile([128, 1 + D], dt.float32, space="PSUM", tag="acc")
            for c in range(K2):
                nc.tensor.matmul(acc2[:], lhsT=oht2[:, c, :], rhs=G2[:, c, :],
                                 start=(c == 0), stop=(c == K2 - 1))
            # ============ epilogue (block pairs): out = elu(U/Z) ============
            if b % 2 == 0:
                Utb = epool.tile([128, 2, 1 + D], dt.float32, tag="Ut")
            nc.vector.tensor_add(Utb[:, b % 2, :], U_t[:], acc2[:])
            if b % 2 == 1:
                zg = epool.tile([128, 2], dt.float32, tag="zg")
                nc.vector.tensor_scalar_max(zg[:], Utb[:, :, D], 1e-30)
                zr = epool.tile([128, 2], dt.float32, tag="zr")
                nc.vector.reciprocal(zr[:], zg[:])
                x = epool.tile([128, 2, D], dt.float32, tag="x")
                nc.vector.tensor_mul(x[:], Utb[:, :, 0:D],
                                     zr[:, :, None].to_broadcast([128, 2, D]))
                mn = epool.tile([128, 2, D], dt.float32, tag="mn")
                nc.vector.tensor_scalar_min(mn[:], x[:], 0.0)
                em = epool.tile([128, 2, D], dt.float32, tag="em")
                nc.scalar.activation(em[:], mn[:],
                                     mybir.ActivationFunctionType.Exp)
                rl = epool.tile([128, 2, D], dt.float32, tag="rl")
                nc.vector.tensor_scalar_max(rl[:], x[:], 0.0)
                nc.vector.scalar_tensor_tensor(
                    out=outstage[:, b - 1:b + 1, :], in0=em[:], scalar=-1.0,
                    op0=mybir.AluOpType.add, in1=rl[:], op1=mybir.AluOpType.add)
            gsp += nm2

        out_v = out_d.rearrange("(b p) d -> p b d", p=128)   # [128, NB, D]
        nc.sync.dma_start(out_v, outstage[:])
    nc.compile()
    return nc


_cache = {}


def _get_program(Kb2, S2, ncellp2):
    key = (tuple(Kb2), S2, tuple(ncellp2), tuple(sorted(_ABL)))
    if key not in _cache:
        _cache[key] = _build(Kb2, S2, ncellp2)
    return _cache[key]


def make_in_maps(A, X, Ws, a):
    """Host-side sharding: returns (nc, in_maps)."""
    import ml_dtypes
    X = np.asarray(X, dtype=np.float32)
    Ws = np.ascontiguousarray(np.asarray(Ws, dtype=np.float32))
    a = np.asarray(a, dtype=np.float32).reshape(2 * D)
    Xbf = X.astype(ml_dtypes.bfloat16)
    WsT = np.ascontiguousarray(Ws.T)
    apair = np.stack([a[D:], a[:D]], axis=1).astype(np.float32)  # [D, 2] = [a2|a1]
    q = np.arange(128)
    CPC = 128 // CELL
    sel16 = (q[:, None] // CPC == np.arange(CELL)[None, :]).astype(np.float32)
    E16 = (q[:, None] % CPC == q[None, :] // CELL).astype(np.float32)
    iotaf = np.tile(np.arange(128, dtype=np.float32)[None, :], (128, 1))
    cores, Kb2, S2, ncellp2 = _prep_edges(A)
    nc = _get_program(Kb2, S2, ncellp2)
    # pad sentinel column: v . (Ws@a2) = TPAD so pad slots get t ~ TPAD and
    # w = exp(leaky(s+t)) ~ e^-58 ~ 0 with no mask op on device
    wsa2 = Ws @ a[D:]
    nrm = float((wsa2 ** 2).sum())
    assert nrm > 1e-8, "degenerate Ws@a2; sentinel padding invalid"
    vpad = (TPAD / nrm) * wsa2
    vpad_bf = vpad.astype(ml_dtypes.bfloat16)
    in_maps = []
    for c in range(NCORES):
        ci = cores[c]
        Xg = Xbf[ci["dsti"]]                                    # [S1+S2, F]
        Xg[:S1][ci["pad1"]] = vpad_bf
        XgT = np.ascontiguousarray(Xg.T)                        # [F, S1+S2]
        XTown = np.ascontiguousarray(Xbf[c * R:(c + 1) * R].T)  # [F, R]
        in_maps.append({
            "XgT": XgT, "XTown": XTown, "Ws": Ws, "WsT": WsT,
            "apair": apair,
            "sel16": sel16, "E16": E16, "iotaf": iotaf,
            "srel2": ci["srel2"], "PMs": ci["PMs"],
        })
    return nc, in_maps


def kernel(A, X, Ws, a):
    nc, in_maps = make_in_maps(A, X, Ws, a)
    res = run_bass_kernel_spmd(nc, in_maps, core_ids=list(range(NCORES)),
                               trace=False)
    return np.concatenate([r["out"] for r in res.results], axis=0)


# revision 27
# speedup vs baseline: 5.4708x; 1.0286x over previous
"""GAT forward kernel for Trainium2 (8 NeuronCores, Bass/Tile).

Reference computation (dense form):
    adj = densify(A); Wh = X @ Ws; e = leaky_relu(Wh@a1 + (Wh@a2).T, 0.15)
    att = softmax(where(adj>0, e, -9e15), axis=1); out = elu(att @ Wh)

Sparse form (only ~524K of 16384^2 entries matter; |e| <= ~16 so softmax
needs no max-subtraction):
    w_e = exp(leaky(s_src + t_dst));  out_i = elu(sum_e w_e Wh_dst / sum_e w_e)

Sharding: rows (softmax queries) split 2048/core across 8 cores.

Two-tier edge layout, host pre-gathers X[dst_e] per slot (contiguous bf16
DMA, no device gather):

Tier 1 (first Q=32 edges of every row): slot (b, q, p) holds edge q of row
(b, p).  One matmul per (b, q) chunk (lhsT = XgT chunk, rhs = [Ws@a2|Ws|1])
gives [t_dst, Wh_dst, 1] for 128 rows at once, ROW-partition.  s_src is a
per-partition scalar broadcast, pad slots are masked with -1e30 before exp,
and the softmax aggregation is a plain DVE reduction over q -- no one-hot
matmuls, no PM machinery.

Tier 2 (edges Q..deg of rows with deg > Q, ~7% of slots): classic row-major
cell layout with per-chunk one-hot (is_equal vs srcrel) aggregation matmuls
and the PM cell trick for per-slot s.  Its acc joins tier 1's U in the
epilogue: out = elu((U1+U2) / (Z1+Z2)).

Host prep pads tiers to cross-core uniform chunk counts so all 8 cores run
the same program (SPMD).
"""
import os
import sys

if "/opt/trn_rl_repo" not in sys.path:
    sys.path.insert(0, "/opt/trn_rl_repo")

_ABL = set(os.environ.get("GAT_ABLATE", "").split(","))

from contextlib import ExitStack

import numpy as np

import concourse.bass as bass
import concourse.tile as tile
from concourse import bacc, mybir
from concourse.bass_utils import run_bass_kernel_spmd

N = 16384          # nodes
F = 128            # input features
D = 64             # embedding dim
NCORES = 8
R = N // NCORES    # rows per core (2048)
NB = R // 128      # row blocks per core (16)
Q = 32             # tier-1 slots per row
S1 = NB * Q * 128  # tier-1 slots per core (65536)
CELL = 8           # tier-2 slots per cell (one src row per cell)
SUB = 11           # A1 chunks per PSUM sub-batch (11*65*4B < 2 banks)
TPAD = -400.0      # pad-slot sentinel: t_pad ~ TPAD so exp(leaky) ~ 0
dt = mybir.dt


# ---------------------------------------------------------------- host prep
def _prep_edges(A):
    """Dedup edges; tier-1 = first Q edges per row in (block, q, row) slot
    order with a -inf pad mask; tier-2 = remaining edges in row-major
    CELL-padded layout, padded to cross-core uniform chunk counts Kb2."""
    src_all = np.asarray(A[0], dtype=np.int64)
    dst_all = np.asarray(A[1], dtype=np.int64)
    keys = np.unique(src_all * N + dst_all)     # dedup + sort by (src, dst)
    src = (keys // N).astype(np.int32)
    dst = (keys % N).astype(np.int32)
    E = len(dst)

    deg = np.bincount(src, minlength=N)
    assert deg.min() > 0, (
        "empty rows present; this kernel assumes every row has >=1 edge"
    )
    row_start = np.concatenate([[0], np.cumsum(deg)])

    # ---- tier 1: dsti1 [NCORES, NB, Q, 128] + pad mask (pads get the
    # sentinel X column so t_pad ~ TPAD and w vanishes without a mask op)
    rows = np.arange(N).reshape(NCORES, NB, 128)
    qs = np.arange(Q)
    pos = row_start[rows][..., None] + qs                 # [C, NB, 128, Q]
    valid = qs[None, None, None, :] < deg[rows][..., None]
    dsti1 = np.where(valid, dst[np.minimum(pos, E - 1)], 0)
    dsti1 = dsti1.transpose(0, 1, 3, 2).astype(np.int32)  # [C, NB, Q, 128]
    pad1 = (~valid).transpose(0, 1, 3, 2).reshape(NCORES, S1)

    # ---- tier 2: excess edges, row-major cells, block buckets
    exc = np.maximum(deg - Q, 0)
    excc = ((exc + CELL - 1) // CELL) * CELL
    slots_cb = excc.reshape(NCORES, NB, 128).sum(axis=2)   # [C, NB]
    Kb2 = np.maximum((slots_cb.max(axis=0) + 127) // 128, 1)   # [NB]
    S2 = int(Kb2.sum()) * 128
    offs2 = np.concatenate([[0], np.cumsum(Kb2)]) * 128
    cells_cb = slots_cb // CELL
    ncell2 = [int(k) * (128 // CELL) for k in Kb2]
    ncellp2 = [((n + 127) // 128) * 128 for n in ncell2]
    cell_offs2 = np.concatenate([[0], np.cumsum(ncellp2)])
    SC2 = int(cell_offs2[-1])

    dsti2 = np.zeros((NCORES, S2), np.int32)
    srel2 = np.full((NCORES, S2), -1.0, np.float32)
    cellsrc2 = np.zeros((NCORES, SC2), np.int16)
    for c in range(NCORES):
        for b in range(NB):
            pos2 = offs2[b]
            for p in range(128):
                r = (c * NB + b) * 128 + p
                d = int(exc[r])
                if d == 0:
                    continue
                lo = row_start[r] + Q
                dsti2[c, pos2:pos2 + d] = dst[lo:lo + d]
                srel2[c, pos2:pos2 + d] = float(p)
                ncw = int(excc[r])
                cbase = cell_offs2[b] + (pos2 - offs2[b]) // CELL
                cellsrc2[c, cbase:cbase + ncw // CELL] = r
                pos2 += ncw
            assert pos2 <= offs2[b + 1]

    import ml_dtypes
    NMTOT = sum(n // 128 for n in ncellp2)
    cores = []
    for c in range(NCORES):
        PMs = np.zeros((128, NMTOT, 128), ml_dtypes.bfloat16)
        g = 0
        for b in range(NB):
            base = (c * NB + b) * 128
            for m in range(ncellp2[b] // 128):
                cs = cellsrc2[c, cell_offs2[b] + m * 128:
                              cell_offs2[b] + (m + 1) * 128]
                rel = cs.astype(np.int64) - base
                vv = (rel >= 0) & (rel < 128)
                PMs[rel[vv], g, np.arange(128)[vv]] = 1.0
                g += 1
        assert g == NMTOT
        cores.append({
            "dsti": np.concatenate([dsti1[c].reshape(-1), dsti2[c]]),
            "pad1": pad1[c],                                     # [S1] bool
            "srel2": srel2[c].reshape(S2 // 128, 128).T.copy(),  # [128, S2/128]
            "PMs": PMs,
        })
    return cores, [int(k) for k in Kb2], S2, [int(x) for x in ncellp2]


# ---------------------------------------------------------------- device IR
def _build(Kb2, S2, ncellp2):
    SC2 = sum(ncellp2)
    NMTOT = SC2 // 128
    ST = S1 + S2
    nc = bacc.Bacc("TRN2", target_bir_lowering=False, debug=False,
                   enable_asserts=False, num_devices=NCORES,
                   num_swdge_queues=4)
    XgT_d = nc.dram_tensor("XgT", [F, ST], dt.bfloat16, kind="ExternalInput").ap()
    XTown_d = nc.dram_tensor("XTown", [F, R], dt.bfloat16, kind="ExternalInput").ap()
    Ws_d = nc.dram_tensor("Ws", [F, D], dt.float32, kind="ExternalInput").ap()
    WsT_d = nc.dram_tensor("WsT", [D, F], dt.float32, kind="ExternalInput").ap()
    apair_d = nc.dram_tensor("apair", [D, 2], dt.float32, kind="ExternalInput").ap()
    PMs_d = nc.dram_tensor("PMs", [128, NMTOT, 128], dt.bfloat16, kind="ExternalInput").ap()
    srel2_d = nc.dram_tensor("srel2", [128, S2 // 128], dt.float32, kind="ExternalInput").ap()
    sel16_d = nc.dram_tensor("sel16", [128, CELL], dt.float32, kind="ExternalInput").ap()
    E16_d = nc.dram_tensor("E16", [128, 128], dt.float32, kind="ExternalInput").ap()
    iotaf_d = nc.dram_tensor("iotaf", [128, 128], dt.float32, kind="ExternalInput").ap()
    out_d = nc.dram_tensor("out", [R, D], dt.float32, kind="ExternalOutput").ap()

    with tile.TileContext(nc) as tc, ExitStack() as ctx:
        cpool = ctx.enter_context(tc.tile_pool(name="const", bufs=1))
        xgpool = ctx.enter_context(tc.tile_pool(name="xg", bufs=3))
        x2pool = ctx.enter_context(tc.tile_pool(name="x2", bufs=3))
        whpool = ctx.enter_context(tc.tile_pool(name="wh", bufs=3))
        Gpool = ctx.enter_context(tc.tile_pool(name="G", bufs=3))
        ohpool = ctx.enter_context(tc.tile_pool(name="oh", bufs=3))
        wpool = ctx.enter_context(tc.tile_pool(name="w", bufs=3))
        epool = ctx.enter_context(tc.tile_pool(name="ep", bufs=3))
        # PSUM budget (8 banks): wt 2x1, acc2 2x1, sc 1, se 2x1
        ps_wt = ctx.enter_context(tc.tile_pool(name="ps_wt", bufs=2, space="PSUM"))
        ps_acc = ctx.enter_context(tc.tile_pool(name="ps_acc", bufs=2, space="PSUM"))
        ps_sc = ctx.enter_context(tc.tile_pool(name="ps_sc", bufs=1, space="PSUM"))
        ps_se = ctx.enter_context(tc.tile_pool(name="ps_se", bufs=1, space="PSUM"))

        # ---- constants
        iota_f = cpool.tile([128, 128], dt.float32)
        nc.sync.dma_start(iota_f[:], iotaf_d)
        ws_t = cpool.tile([F, D], dt.float32)
        nc.sync.dma_start(ws_t[:], Ws_d)
        wsT_t = cpool.tile([D, F], dt.float32)
        nc.sync.dma_start(wsT_t[:], WsT_d)
        apair_t = cpool.tile([D, 2], dt.float32)
        nc.sync.dma_start(apair_t[:], apair_d)
        srel2_t = cpool.tile([128, S2 // 128], dt.float32)
        nc.sync.dma_start(srel2_t[:], srel2_d)
        sel16_t = cpool.tile([128, CELL], dt.float32)
        nc.sync.dma_start(sel16_t[:], sel16_d)
        E16_t = cpool.tile([128, 128], dt.float32)
        nc.sync.dma_start(E16_t[:], E16_d)
        PMs_t = cpool.tile([128, NMTOT, 128], dt.bfloat16)
        nc.sync.dma_start(PMs_t[:], PMs_d)
        xtown_t = cpool.tile([F, NB, 128], dt.bfloat16)
        nc.scalar.dma_start(xtown_t[:], XTown_d.rearrange("f (b p) -> f b p", p=128))

        # Wse = [Ws@a2 | Ws] bf16: one 65-col rhs so each A1 matmul yields
        # [t_dst, Wh_dst] per slot.  Wsa1 bf16 for the s matmuls.
        wsa_ps = ps_sc.tile([128, 2], dt.float32, space="PSUM", tag="sc")
        nc.tensor.matmul(wsa_ps[:], lhsT=wsT_t[:], rhs=apair_t[:],
                         start=True, stop=True)
        Wse = cpool.tile([F, 1 + D], dt.bfloat16)
        nc.vector.tensor_copy(Wse[:, 0:1], wsa_ps[:, 0:1])
        nc.vector.tensor_copy(Wse[:, 1:1 + D], ws_t[:])
        wsa1_t = cpool.tile([F, 1], dt.bfloat16)
        nc.vector.tensor_copy(wsa1_t[:], wsa_ps[:, 1:2])

        # ---- s for own rows: s[r] = X[r] @ Ws @ a1, per block -> [128, NB]
        s_ps = ps_se.tile([128, NB], dt.float32, space="PSUM", tag="se")
        for b in range(NB):
            nc.tensor.matmul(s_ps[:, b:b + 1], lhsT=xtown_t[:, b, :],
                             rhs=wsa1_t[:], start=True, stop=True)
        sloc = cpool.tile([128, NB], dt.float32)
        nc.vector.tensor_copy(sloc[:], s_ps[:])
        # hi/lo bf16 split so the tier-2 PM matmuls stay near-f32 exact
        sloc_hl = cpool.tile([128, NB, 2], dt.bfloat16)
        nc.vector.tensor_copy(sloc_hl[:, :, 0], s_ps[:])
        nc.vector.tensor_sub(sloc_hl[:, :, 1], s_ps[:], sloc_hl[:, :, 0])

        # ---- main loop: per 128-row block
        sl2 = [0]
        for b in range(NB):
            sl2.append(sl2[-1] + Kb2[b])
        XgT1_v = XgT_d[:, 0:S1].rearrange("f (b q p) -> f b q p", q=Q, p=128)
        outstage = cpool.tile([128, NB, D], dt.float32)
        gsp = 0          # global PM span index
        for b in range(NB):
            K2 = Kb2[b]
            nm2 = ncellp2[b] // 128
            # ================= tier 1 =================
            xg1 = xgpool.tile([128, Q, 128], dt.bfloat16)
            nc.sync.dma_start(xg1[:], XgT1_v[:, b])
            t_f = wpool.tile([128, Q], dt.float32, tag="tf")
            whp = whpool.tile([128, 1 + D, Q], dt.bfloat16)   # [Wh|1][d, q]
            nc.vector.memset(whp[:, D, :], 1.0)
            # sub-batches through a 2-bank PSUM tile: 7 chunks of 65 per bank
            # (matmul outputs must stay inside one 512-f32 bank)
            c0 = 0
            for nb, ns in ((2, 7), (2, 7), (1, 4)):
                n = nb * ns
                wt_ps = ps_wt.tile([128, 2, 512], dt.float32,
                                   space="PSUM", tag="wt")
                for j in range(n):
                    nc.tensor.matmul(
                        wt_ps[:, j // ns, (j % ns) * 65:(j % ns) * 65 + 65],
                        lhsT=xg1[:, c0 + j, :],
                        rhs=Wse[:], start=True, stop=True)
                wt_v = wt_ps[:, 0:nb, 0:ns * 65].rearrange(
                    "p b (s d) -> p b s d", d=65)
                nc.vector.tensor_copy(
                    t_f[:, c0:c0 + n].rearrange("p (b s) -> p b s", s=ns),
                    wt_v[:, :, :, 0])
                nc.scalar.activation(
                    whp[:, 0:D, c0:c0 + n].rearrange("p d (b s) -> p d b s", s=ns),
                    wt_v[:, :, :, 1:].rearrange("p b s d -> p d b s"),
                    mybir.ActivationFunctionType.Copy)
                c0 += n
            # w = exp(leaky(s + t)); s+t fused into the ACT exp's bias arg,
            # leaky via max(0.15e, e) on DVE
            e_t = wpool.tile([128, Q], dt.float32, tag="e")
            nc.vector.tensor_scalar_add(e_t[:], t_f[:], sloc[:, b:b + 1])
            lk = wpool.tile([128, Q], dt.float32, tag="lk")
            nc.vector.scalar_tensor_tensor(
                out=lk[:], in0=e_t[:], scalar=0.15, op0=mybir.AluOpType.mult,
                in1=e_t[:], op1=mybir.AluOpType.max)
            w_t = wpool.tile([128, Q], dt.bfloat16, tag="wt")
            nc.scalar.activation(w_t[:], lk[:], mybir.ActivationFunctionType.Exp)
            # G[d, q] = w[q] * [Wh, 1][d, q]; U[d] = sum_q G[d, q]
            G = Gpool.tile([128, 1 + D, Q], dt.bfloat16)
            nc.vector.tensor_mul(G[:], whp[:],
                                 w_t[:, None, :].to_broadcast([128, 1 + D, Q]))
            # two pairwise bf16 folds quarter the f32-out reduce's 1x work
            H = Gpool.tile([128, 1 + D, Q // 2], dt.bfloat16, tag="H")
            nc.vector.tensor_add(H[:], G[:, :, 0:Q // 2], G[:, :, Q // 2:Q])
            H2 = Gpool.tile([128, 1 + D, Q // 4], dt.bfloat16, tag="H2")
            nc.vector.tensor_add(H2[:], H[:, :, 0:Q // 4], H[:, :, Q // 4:Q // 2])
            U_t = epool.tile([128, 1 + D], dt.float32, tag="U")
            nc.vector.reduce_sum(U_t[:], H2[:], axis=mybir.AxisListType.X)
            # ================= tier 2 =================
            lo2 = S1 + sl2[b] * 128
            xg2 = x2pool.tile([128, K2, 128], dt.bfloat16)
            nc.sync.dma_start(
                xg2[:], XgT_d[:, lo2:lo2 + K2 * 128]
                .rearrange("f (k p) -> f k p", p=128))
            t2_f = wpool.tile([128, K2], dt.float32, tag="t2")
            wh2 = whpool.tile([128, K2, 1 + D], dt.bfloat16)
            for c0 in range(0, K2, 7):
                c1 = min(c0 + 7, K2)
                wt_ps = ps_wt.tile([128, 2, 512], dt.float32,
                                   space="PSUM", tag="wt")
                for j in range(c1 - c0):
                    nc.tensor.matmul(wt_ps[:, 0, j * 65:j * 65 + 65],
                                     lhsT=xg2[:, c0 + j, :],
                                     rhs=Wse[:], start=True, stop=True)
                wt2_v = wt_ps[:, 0, 0:(c1 - c0) * 65].rearrange(
                    "p (s d) -> p s d", d=65)
                nc.vector.tensor_copy(t2_f[:, c0:c1], wt2_v[:, :, 0])
                nc.scalar.activation(
                    wh2[:, c0:c1, 0:D], wt2_v[:, :, 1:],
                    mybir.ActivationFunctionType.Copy)
            # one-hot of srcrel vs row-in-block
            oht2 = ohpool.tile([128, K2, 128], dt.bfloat16)
            nc.vector.tensor_tensor(
                out=oht2[:],
                in0=iota_f[:, None, :].to_broadcast([128, K2, 128]),
                in1=srel2_t[:, sl2[b]:sl2[b] + K2, None]
                    .to_broadcast([128, K2, 128]),
                op=mybir.AluOpType.is_equal)
            # s per cell via PM one-hot matmuls, then expand cells -> slots
            sc_ps = ps_sc.tile([128, nm2], dt.float32, space="PSUM", tag="sc")
            for m in range(nm2):
                nc.tensor.matmul(sc_ps[:, m:m + 1], lhsT=PMs_t[:, gsp + m, :],
                                 rhs=sloc_hl[:, b, 0:1], start=True, stop=False)
                nc.tensor.matmul(sc_ps[:, m:m + 1], lhsT=PMs_t[:, gsp + m, :],
                                 rhs=sloc_hl[:, b, 1:2], start=False, stop=True)
            se_ps = ps_se.tile([128, nm2 * CELL], dt.float32, space="PSUM", tag="se")
            for m in range(nm2):
                rhsm = wpool.tile([128, CELL], dt.float32, tag="rhsm")
                nc.vector.tensor_mul(
                    rhsm[:], sel16_t[:],
                    sc_ps[:, m:m + 1].to_broadcast([128, CELL]))
                nc.tensor.matmul(se_ps[:, m * CELL:(m + 1) * CELL],
                                 lhsT=E16_t[:], rhs=rhsm[:],
                                 start=True, stop=True)
            e2 = wpool.tile([128, K2], dt.float32, tag="e2")
            nc.vector.tensor_add(e2[:], se_ps[:, 0:K2], t2_f[:])
            lk2 = wpool.tile([128, K2], dt.float32, tag="lk2")
            nc.vector.scalar_tensor_tensor(
                out=lk2[:], in0=e2[:], scalar=0.15, op0=mybir.AluOpType.mult,
                in1=e2[:], op1=mybir.AluOpType.max)
            w2 = wpool.tile([128, K2], dt.bfloat16, tag="w2")
            nc.scalar.activation(w2[:], lk2[:], mybir.ActivationFunctionType.Exp)
            G2 = Gpool.tile([128, K2, 1 + D], dt.bfloat16)
            nc.vector.tensor_mul(G2[:, :, 0:D], wh2[:, :, 0:D],
                                 w2[:, :, None].to_broadcast([128, K2, D]))
            nc.vector.tensor_copy(G2[:, :, D], w2[:])
            acc2 = ps_acc.tile([128, 1 + D], dt.float32, space="PSUM", tag="acc")
            for c in range(K2):
                nc.tensor.matmul(acc2[:], lhsT=oht2[:, c, :], rhs=G2[:, c, :],
                                 start=(c == 0), stop=(c == K2 - 1))
            # ============ epilogue (block pairs): out = elu(U/Z) ============
            if b % 2 == 0:
                Utb = epool.tile([128, 2, 1 + D], dt.float32, tag="Ut")
            nc.vector.tensor_add(Utb[:, b % 2, :], U_t[:], acc2[:])
            if b % 2 == 1:
                zg = epool.tile([128, 2], dt.float32, tag="zg")
                nc.vector.tensor_scalar_max(zg[:], Utb[:, :, D], 1e-30)
                zr = epool.tile([128, 2], dt.float32, tag="zr")
                nc.vector.reciprocal(zr[:], zg[:])
                x = epool.tile([128, 2, D], dt.float32, tag="x")
                nc.vector.tensor_mul(x[:], Utb[:, :, 0:D],
                                     zr[:, :, None].to_broadcast([128, 2, D]))
                mn = epool.tile([128, 2, D], dt.float32, tag="mn")
                nc.vector.tensor_scalar_min(mn[:], x[:], 0.0)
                em = epool.tile([128, 2, D], dt.float32, tag="em")
                nc.scalar.activation(em[:], mn[:],
                                     mybir.ActivationFunctionType.Exp)
                rl = epool.tile([128, 2, D], dt.float32, tag="rl")
                nc.vector.tensor_scalar_max(rl[:], x[:], 0.0)
                nc.vector.scalar_tensor_tensor(
                    out=outstage[:, b - 1:b + 1, :], in0=em[:], scalar=-1.0,
                    op0=mybir.AluOpType.add, in1=rl[:], op1=mybir.AluOpType.add)
            gsp += nm2

        out_v = out_d.rearrange("(b p) d -> p b d", p=128)   # [128, NB, D]
        nc.sync.dma_start(out_v, outstage[:])
    nc.compile()
    return nc


_cache = {}


def _get_program(Kb2, S2, ncellp2):
    key = (tuple(Kb2), S2, tuple(ncellp2), tuple(sorted(_ABL)))
    if key not in _cache:
        _cache[key] = _build(Kb2, S2, ncellp2)
    return _cache[key]


def make_in_maps(A, X, Ws, a):
    """Host-side sharding: returns (nc, in_maps)."""
    import ml_dtypes
    X = np.asarray(X, dtype=np.float32)
    Ws = np.ascontiguousarray(np.asarray(Ws, dtype=np.float32))
    a = np.asarray(a, dtype=np.float32).reshape(2 * D)
    Xbf = X.astype(ml_dtypes.bfloat16)
    WsT = np.ascontiguousarray(Ws.T)
    apair = np.stack([a[D:], a[:D]], axis=1).astype(np.float32)  # [D, 2] = [a2|a1]
    q = np.arange(128)
    CPC = 128 // CELL
    sel16 = (q[:, None] // CPC == np.arange(CELL)[None, :]).astype(np.float32)
    E16 = (q[:, None] % CPC == q[None, :] // CELL).astype(np.float32)
    iotaf = np.tile(np.arange(128, dtype=np.float32)[None, :], (128, 1))
    cores, Kb2, S2, ncellp2 = _prep_edges(A)
    nc = _get_program(Kb2, S2, ncellp2)
    # pad sentinel column: v . (Ws@a2) = TPAD so pad slots get t ~ TPAD and
    # w = exp(leaky(s+t)) ~ e^-58 ~ 0 with no mask op on device
    wsa2 = Ws @ a[D:]
    nrm = float((wsa2 ** 2).sum())
    assert nrm > 1e-8, "degenerate Ws@a2; sentinel padding invalid"
    vpad = (TPAD / nrm) * wsa2
    vpad_bf = vpad.astype(ml_dtypes.bfloat16)
    in_maps = []
    for c in range(NCORES):
        ci = cores[c]
        Xg = Xbf[ci["dsti"]]                                    # [S1+S2, F]
        Xg[:S1][ci["pad1"]] = vpad_bf
        XgT = np.ascontiguousarray(Xg.T)                        # [F, S1+S2]
        XTown = np.ascontiguousarray(Xbf[c * R:(c + 1) * R].T)  # [F, R]
        in_maps.append({
            "XgT": XgT, "XTown": XTown, "Ws": Ws, "WsT": WsT,
            "apair": apair,
            "sel16": sel16, "E16": E16, "iotaf": iotaf,
            "srel2": ci["srel2"], "PMs": ci["PMs"],
        })
    return nc, in_maps


def kernel(A, X, Ws, a):
    nc, in_maps = make_in_maps(A, X, Ws, a)
    res = run_bass_kernel_spmd(nc, in_maps, core_ids=list(range(NCORES)),
                               trace=False)
    return np.concatenate([r["out"] for r in res.results], axis=0)
